# revision 45
# baseline (speedup 1.0000x reference)
"""MultiHeadAttention forward on 8 Trainium2 NeuronCores.

Sharding (Megatron-style tensor parallel x data parallel):
  core c (0..7): batch b = c // 4, head group g = c % 4 (4 of 16 heads).
  Wq/Wk/Wv column-sharded ([1024, 256] per core), Wo row-sharded
  ([256, 1024] per core). Each core computes a partial output
  [S, D] = attn(heads g) @ Wo_rows; the host sums the 4 partials per
  batch and adds bo (the "all-reduce" runs on host since full outputs
  are gathered anyway).

Projections/attnV/O run in bf16 (inputs converted on host; f32 PSUM
accumulate), halving HBM traffic vs f32r at the same PE rate. The
scores matmul runs in fp8-e4m3 DoubleRow perf mode (2x PE rate): K^T
and Q^T are drained from their projection psums into a [32, 2, s]
layout (d = 32*i + p) so each head's QK^T contracts as two 32-row
halves summed in the PE. Measured end-to-end rel err ~7.8e-3 vs the
2e-2 gate.

Schedule notes (the three serial chains that matter):
  - ACT runs the 128 softmax exps (~133us serial) plus a few psum
    drains placed in its natural stalls; it is kept fed from ~22us on.
  - Projection psums add biases via ones-row matmuls INSIDE the psum
    accumulation so the psum->SBUF drains are pure copies with no DMA
    dependency (the conservative DMA-queue semaphore encoding would
    otherwise stall each drain on every earlier DMA on its queue).
  - PE is warmed up on junk matmuls during the first DMA so Kb0 runs
    at full clock; K streams block-major so the first scores chunk
    only needs K-block 0 + Q-block 0.
  - attnV trails scores by ~4 blocks (e2 pool bufs=6); softmax
    normalization = DVE reciprocal + GPSIMD partition-broadcast (Pool
    engine, otherwise idle) + one DVE multiply.
  - O projection drains via DVE mid-kernel and via ACT for the last
    q-block (ACT is idle once the exps finish); host sums the 4
    row-shard partials per batch and adds bo.
"""

import math
from contextlib import ExitStack

import numpy as np
import ml_dtypes

import concourse.bass as bass
import concourse.mybir as mybir
import concourse.tile as tile
from concourse import bacc
from concourse.bass_utils import run_bass_kernel_spmd

P = 128
B, S, D, H = 2, 2048, 1024, 16
NCORES = 8
GROUPS = NCORES // B          # 4 head-groups
HPC = H // GROUPS             # 4 heads per core
DK = D // H                   # 64
CPC = HPC * DK                # 256 cols per core
NP = CPC // P                 # 2 head pairs per core
DC = D // P                   # 8 contraction chunks over D
QB = 512                      # q block (matmul moving free dim)

F32 = mybir.dt.float32
BF = mybir.dt.bfloat16
F8 = mybir.dt.float8e4


def build_program(seq=S):
    KT = seq // P             # k tiles
    NJ = seq // QB            # q blocks
    K2 = KT // 2              # two score k-tiles share one psum / exp op
    KT2 = KT // 2             # k-tile pairs for the V projection
    inv_sqrt_s = 1.0 / math.sqrt(S)  # reference scales by sqrt(full S)

    nc = bacc.Bacc("TRN2", target_bir_lowering=False, debug=False,
                   num_devices=NCORES)
    xqT = nc.declare_dram_parameter("xqT", [D, seq], BF, isOutput=False)
    xkT = nc.declare_dram_parameter("xkT", [D, seq], BF, isOutput=False)
    xvT = nc.declare_dram_parameter("xvT", [D, seq], BF, isOutput=False)
    wq = nc.declare_dram_parameter("wq", [D, CPC], BF, isOutput=False)
    wk = nc.declare_dram_parameter("wk", [D, CPC], BF, isOutput=False)
    wv = nc.declare_dram_parameter("wv", [D, CPC], BF, isOutput=False)
    wo = nc.declare_dram_parameter("wo", [CPC, D], BF, isOutput=False)
    bq = nc.declare_dram_parameter("bq", [1, CPC], BF, isOutput=False)
    bk = nc.declare_dram_parameter("bk", [1, CPC], BF, isOutput=False)
    bv = nc.declare_dram_parameter("bv", [1, CPC], BF, isOutput=False)
    ones_row = nc.declare_dram_parameter("ones_row", [1, QB], BF,
                                         isOutput=False)
    ones_fr = nc.declare_dram_parameter("ones_fr", [1, DK],
                                        mybir.dt.float32r, isOutput=False)
    vones = nc.declare_dram_parameter("vones", [P, KT * HPC], BF,
                                      isOutput=False)
    out = nc.declare_dram_parameter("out", [seq, D], BF, isOutput=True)

    xqT_r = xqT.rearrange("(dc p) s -> p dc s", p=P)
    xkT_r = xkT.rearrange("(dc p) s -> p dc s", p=P)
    xvT_r = xvT.rearrange("(dc p) s -> p dc s", p=P)

    with tile.TileContext(nc) as tc, ExitStack() as st:
        consts = st.enter_context(tc.tile_pool(name="consts", bufs=1))
        bq_sb = consts.tile([1, CPC], BF)
        bk_sb = consts.tile([1, CPC], BF)
        bv_sb = consts.tile([1, CPC], BF)
        ones_sb = consts.tile([1, QB], BF)
        ones_fr_sb = consts.tile([1, DK], mybir.dt.float32r)

        # Persistent activations. K^T/Q^T live in fp8 with the
        # DoubleRow layout: head h on partitions 32h..32h+31, free dims
        # (i, s) where d = 32 i + p — so the scores matmul runs in fp8
        # DoubleRow perf mode at 0.5 cycles/row (2x PE rate).
        # matmul operands need base partition in {0, 32, 64}: heads
        # 0-2 share tile 0 at bases 0/32/64, head 3 gets tile 1 base 0.
        kt_f8 = [consts.tile([P, 2, seq], F8, name=f"kt_f8_{t}")
                 for t in range(2)]
        qt_f8 = [[consts.tile([P, 2, QB], F8, name=f"qt_f8_{j}_{t}")
                  for t in range(2)] for j in range(NJ)]

        def hrow(h):
            t, b = (0, 32 * h) if h < 3 else (1, 0)
            return t, slice(b, b + 32)
        v_sb = consts.tile([P, KT, HPC, DK + 1], BF)
        at_j = [consts.tile([P, NP, QB], BF, name=f"at_j{j}")
                for j in range(NJ)]
        wo_sb = consts.tile([P, NP, D], BF)

        # Warm-up exp so the activation-table load happens during the
        # initial DMA instead of right before the first scores exp.
        warm = consts.tile([1, 1], F32)
        # biases are added inside the psum accumulation via a ones-row
        # matmul (like V) so the psum drains carry NO DMA dependency:
        # the conservative DMA-queue semaphore encoding would otherwise
        # stall each drain on every earlier-enqueued DMA on that queue.

        wqp = st.enter_context(tc.tile_pool(name="wqp", bufs=1))
        xqp = st.enter_context(tc.tile_pool(name="xqp", bufs=2))
        wq_sb = wqp.tile([P, DC, CPC], BF)
        wq_r = wq.rearrange("(dc p) c -> p dc c", p=P)

        def emit_qblock(j, pool, defer_drains=False, split_dma=False):
            xt = xqp.tile([P, DC, QB], BF, tag="xq")
            qsl = xqT_r[:, :, j * QB:(j + 1) * QB]
            if split_dma:
                # halve the first block's DMA so dc 0-3 matmuls start a
                # transfer earlier during the serial bootstrap
                nc.sync.dma_start(xt[:, 0:DC // 2], qsl[:, 0:DC // 2])
                nc.sync.dma_start(xt[:, DC // 2:], qsl[:, DC // 2:])
            else:
                nc.sync.dma_start(xt[:], qsl)
            ps = [pool.tile([P, QB], F32, tag="k", name=f"psq_{j}_{pi}")
                  for pi in range(NP)]
            for dc in range(DC):
                for pi in range(NP):
                    nc.tensor.matmul(
                        ps[pi][:],
                        wq_sb[:, dc, pi * P:(pi + 1) * P],
                        xt[:, dc],
                        start=(dc == 0), stop=False,
                    )
            for pi in range(NP):
                nc.tensor.matmul(  # += bq^T @ ones  (bias add)
                    ps[pi][:], bq_sb[:, pi * P:(pi + 1) * P], ones_sb[:],
                    start=False, stop=True,
                )

            def drains(pis):
                for pi in pis:
                    for hp in range(2):
                        for i in range(2):
                            r = hp * 64 + 32 * i
                            t, rows = hrow(2 * pi + hp)
                            nc.vector.tensor_copy(
                                qt_f8[j][t][rows, i, :],
                                ps[pi][r:r + 32, :])
            if defer_drains == "p1":
                drains([0])
                return lambda: drains([1])
            if defer_drains:
                return lambda: drains(range(NP))
            drains(range(NP))

        # K projection, block-major like Q (one [P, DC, QB] DMA per
        # block) so kt_p columns drain progressively and the first
        # scores exps can start right after K's matmuls.
        xkp = st.enter_context(tc.tile_pool(name="xkp", bufs=2))
        wkp = st.enter_context(tc.tile_pool(name="wkp", bufs=1))
        wk_sb = wkp.tile([P, DC, CPC], BF)
        wk_r = wk.rearrange("(dc p) c -> p dc c", p=P)

        def emit_kblock(qc, pool, act_p1=False, split_dma=False,
                        defer_p0h1=False):
            xt = xkp.tile([P, DC, QB], BF, tag="xk")
            ksl = xkT_r[:, :, qc * QB:(qc + 1) * QB]
            if split_dma:
                nc.sync.dma_start(xt[:, 0:DC // 2], ksl[:, 0:DC // 2])
                nc.sync.dma_start(xt[:, DC // 2:], ksl[:, DC // 2:])
            else:
                nc.sync.dma_start(xt[:], ksl)
            ps = [pool.tile([P, QB], F32, tag="k", name=f"psk_{qc}_{pi}")
                  for pi in range(NP)]
            for dc in range(DC):
                for pi in range(NP):
                    nc.tensor.matmul(
                        ps[pi][:],
                        wk_sb[:, dc, pi * P:(pi + 1) * P],
                        xt[:, dc],
                        start=(dc == 0), stop=False,
                    )
            for pi in range(NP):
                nc.tensor.matmul(  # += bk^T @ ones  (bias add)
                    ps[pi][:], bk_sb[:, pi * P:(pi + 1) * P], ones_sb[:],
                    start=False, stop=True,
                )
            def kdrain(pi, hp):
                for i in range(2):
                    r = hp * 64 + 32 * i
                    t, rows = hrow(2 * pi + hp)
                    dst = kt_f8[t][rows, i, qc * QB:(qc + 1) * QB]
                    if pi == 1 and act_p1:
                        # heads 2/3 drain on ACT: they are needed a
                        # whole exp-block later and fill the early
                        # ACT stalls, lightening the serial DVE queue
                        nc.scalar.copy(dst, ps[pi][r:r + 32, :])
                    else:
                        nc.vector.tensor_copy(dst, ps[pi][r:r + 32, :])
            kdrain(0, 0)
            if not defer_p0h1:
                kdrain(0, 1)
            kdrain(1, 0)
            kdrain(1, 1)
            if defer_p0h1:
                return lambda: kdrain(0, 1)

        # ---- attention pipeline pieces ----
        ep = st.enter_context(tc.tile_pool(name="epool", bufs=6))
        rp = st.enter_context(tc.tile_pool(name="rpool", bufs=2))
        op = st.enter_context(tc.tile_pool(name="opool", bufs=8))

        def emit_scores(j, h, e2, k2s, pss_p):
            t, rows = hrow(h)
            for k2 in k2s:
                pss = pss_p.tile([P, 2 * QB], F32, tag="s",
                                 name=f"pss_{j}_{h}_{k2}")
                for half in range(2):
                    kt = 2 * k2 + half
                    nc.tensor.matmul(
                        pss[:, half * QB:(half + 1) * QB],
                        kt_f8[t][rows, :, kt * P:(kt + 1) * P],
                        qt_f8[j][t][rows, :, :],
                        start=True, stop=True,
                        perf_mode=mybir.MatmulPerfMode.DoubleRow,
                    )
                nc.scalar.activation(
                    e2[:, k2], pss[:],
                    mybir.ActivationFunctionType.Exp,
                    scale=inv_sqrt_s,
                )

        e2t = {}

        def Sblk(j, h, k2s=None, pss_pool=None):
            if k2s is None or k2s[0] == 0:
                e2 = ep.tile([P, K2, 2 * QB], BF, tag="E",
                             name=f"e2_{j}_{h}")
                e2t[(j, h)] = e2
            emit_scores(j, h, e2t[(j, h)], k2s or range(K2),
                        pss_pool or pss_p)

        # ---- V projection (kt-pair-major, one psum bank per pair) ----
        v_stack = ExitStack()
        q_stack = ExitStack()

        def emit_vblock(kt2, xvp, psv_p):
            xt = xvp.tile([P, DC, 2 * P], BF, tag="xv")
            nc.sync.dma_start(
                xt[:], xvT_r[:, :, kt2 * 2 * P:(kt2 + 1) * 2 * P])
            psv = psv_p.tile([P, 2, CPC], F32, tag="v", name=f"psv_{kt2}")
            for dc in range(DC):
                for half in range(2):
                    nc.tensor.matmul(
                        psv[:, half],
                        xt[:, dc, half * P:(half + 1) * P],
                        wv_sb[:, dc],
                        start=(dc == 0 and half == 0), stop=False,
                    )
            for half in range(2):
                nc.tensor.matmul(  # += ones^T @ bv  (bias add)
                    psv[:, half], ones_sb[:, :P], bv_sb[:],
                    start=False, stop=(half == 1),
                )
            for half in range(2):
                nc.vector.tensor_copy(
                    v_sb[:, 2 * kt2 + half, :, 0:DK],
                    psv[:, half].rearrange("p (h d) -> p h d", h=HPC),
                )

        def emit_attnv(j, h, e2, psa_p, pe_bcast=None):
            hp, hj = h % 2, h // 2
            prow = slice(hp * DK, (hp + 1) * DK)
            psa = psa_p.tile([P, QB], F32, tag="a", name=f"psa_{j}_{h}")
            for kt in range(KT):
                nc.tensor.matmul(
                    psa[:DK + 1],
                    v_sb[:, kt, h, :],
                    e2[:, kt // 2, (kt % 2) * QB:(kt % 2 + 1) * QB],
                    start=(kt == 0), stop=(kt == KT - 1),
                )
            # softmax denominator is psa row DK; normalize via DVE recip +
            # GPSIMD partition-broadcast (Pool engine) + DVE multiply.
            # On the last block the Pool round-trip is on the critical
            # tail: broadcast via a PE matmul (f32r, exact) into a
            # borrowed psum bank instead.
            if pe_bcast is not None:
                rc = rp.tile([1, QB], mybir.dt.float32r, tag="rcr",
                             bufs=1)
                with nc.allow_low_precision(
                        reason="f32r reciprocal for matmul bcast"):
                    nc.vector.reciprocal(rc[:], psa[DK:DK + 1, :])
                prc = pe_bcast.tile([P, QB], F32, tag="o",
                                    name=f"prc_{j}_{h}")
                nc.tensor.matmul(prc[:DK], ones_fr_sb[:], rc[:],
                                 start=True, stop=True)
                # tensor_tensor may read only one PSUM operand: stage
                # psa through SBUF (overlaps the PE broadcast matmul)
                atmp = rp.tile([DK, QB], F32, tag="prc")
                nc.vector.tensor_copy(atmp[:], psa[:DK])
                nc.vector.tensor_tensor(
                    at_j[j][prow, hj, :], atmp[:], prc[:DK],
                    mybir.AluOpType.mult,
                )
                return
            rc = rp.tile([1, QB], F32, tag="rc")
            nc.vector.reciprocal(rc[:], psa[DK:DK + 1, :])
            prc = rp.tile([DK, QB], F32, tag="prc")
            nc.gpsimd.partition_broadcast(prc[:], rc[:])
            nc.vector.tensor_tensor(
                at_j[j][prow, hj, :], psa[:DK], prc[:],
                mybir.AluOpType.mult,
            )

        def emit_oproj(j, pso_p, act_copy=False, chunks=None,
                       alt_pool=None):
            for ql in (range(QB // P) if chunks is None else chunks):
                qt0 = j * (QB // P) + ql
                for nh in range(D // QB):
                    o_sb = op.tile([P, QB], BF, tag="o_sb",
                                   name=f"osb_{qt0}_{nh}")
                    # tail block: borrow the attnV psum banks (drained
                    # by then) so four banks rotate instead of two
                    pp = alt_pool if (alt_pool is not None
                                      and (ql * 2 + nh) % 2 == 1) else pso_p
                    pso = pp.tile([P, QB], F32,
                                  tag="a" if pp is alt_pool else "o",
                                  name=f"pso_{qt0}_{nh}")
                    for dj in range(NP):
                        nc.tensor.matmul(
                            pso[:],
                            at_j[j][:, dj, ql * P:(ql + 1) * P],
                            wo_sb[:, dj, nh * QB:(nh + 1) * QB],
                            start=(dj == 0), stop=(dj == NP - 1),
                        )
                    if act_copy and (ql * 2 + nh) % 2 == 0:
                        # tail block: alternate ACT/DVE copies (both are
                        # idle once the exps and norms finish)
                        nc.scalar.copy(o_sb[:], pso[:])
                    else:
                        nc.vector.tensor_copy(o_sb[:], pso[:])
                    nc.sync.dma_start(
                        out[qt0 * P:(qt0 + 1) * P,
                            nh * QB:(nh + 1) * QB],
                        o_sb[:],
                    )

        # ---- interleaved emission schedule ----
        # PE queue order == execution order. Scores blocks are
        # ACT-throttled (~8.3us each via the pss double-buffer), so the
        # Q/K/V projection matmuls placed between them execute when
        # their DMA lands, filling PE gaps. attnV trails scores by 3-4
        # blocks (e2 pool bufs=5).
        pss_p = st.enter_context(tc.tile_pool(name="ps_s", bufs=2,
                                              space="PSUM"))
        with tc.tile_pool(name="ps_kq", bufs=4, space="PSUM") as ps_k:
            nc.sync.dma_start(wk_sb[:], wk_r)
            # PE warm-up: ramp the tensor engine to full clock on junk
            # matmuls over wk while xkb0 streams, and prefetch the exp
            # activation table, so Kb0 runs at full rate immediately.
            nc.sync.dma_start(bk_sb[:], bk[:])
            nc.sync.dma_start(ones_sb[:], ones_row[:])
            wps = pss_p.tile([P, 2 * QB], F32, tag="s", name="warm_ps")
            for w in range(16):
                nc.tensor.matmul(wps[:, :CPC], wk_sb[:, 0, :P],
                                 wk_sb[:, 0, :], start=True, stop=True)
            nc.scalar.activation(warm[:], wk_sb[0:1, 0:1, 0:1],
                                 mybir.ActivationFunctionType.Exp)
            emit_kblock(0, ps_k, act_p1=True)
            nc.sync.dma_start(wq_sb[:], wq_r)
            nc.sync.dma_start(bq_sb[:], bq[:])
            q0_p1 = emit_qblock(0, ps_k, defer_drains="p1")
            Sblk(0, 0, range(0, 2))
            emit_kblock(1, ps_k, act_p1=True)
            q0_p1()
            Sblk(0, 0, range(2, 4))
            k2_h1 = emit_kblock(2, ps_k, act_p1=True, defer_p0h1=True)
            Sblk(0, 0, range(4, 6))
            k3_h1 = emit_kblock(3, ps_k, act_p1=True, defer_p0h1=True)
            Sblk(0, 0, range(6, 8))
            emit_qblock(1, ps_k)
            k2_h1()
            k3_h1()
        q_psum = q_stack.enter_context(
            tc.tile_pool(name="ps_q", bufs=2, space="PSUM"))
        Sblk(0, 1)
        emit_qblock(2, q_psum)
        Sblk(0, 2)
        q3_drains = emit_qblock(3, q_psum, defer_drains=True)
        xvp = v_stack.enter_context(tc.tile_pool(name="xvp", bufs=2))
        wvp = v_stack.enter_context(tc.tile_pool(name="wvp", bufs=1))
        psv_p = v_stack.enter_context(
            tc.tile_pool(name="ps_v", bufs=2, space="PSUM"))
        wv_sb = wvp.tile([P, DC, CPC], BF)
        nc.sync.dma_start(wv_sb[:], wv.rearrange("(dc p) c -> p dc c", p=P))
        nc.sync.dma_start(bv_sb[:], bv[:])
        nc.sync.dma_start(ones_sb[:], ones_row[:])
        emit_vblock(0, xvp, psv_p)
        emit_vblock(1, xvp, psv_p)
        Sblk(0, 3)
        emit_vblock(2, xvp, psv_p)
        emit_vblock(3, xvp, psv_p)
        Sblk(1, 0)
        emit_vblock(4, xvp, psv_p)
        emit_vblock(5, xvp, psv_p)
        Sblk(1, 1)
        emit_vblock(6, xvp, psv_p)
        emit_vblock(7, xvp, psv_p)
        with nc.allow_non_contiguous_dma(
                reason="one-time 16KB ones-column init"):
            nc.sync.dma_start(
                v_sb[:, :, :, DK:DK + 1],
                vones.rearrange("p (kt h) -> p kt h",
                                kt=KT, h=HPC)[:, :, :, None],
            )
        nc.sync.dma_start(wo_sb[:],
                          wo.rearrange("(dj p) n -> p dj n", p=P))
        nc.sync.dma_start(ones_fr_sb[:], ones_fr[:])
        q3_drains()
        v_stack.close()
        q_stack.close()
        psa_p = st.enter_context(tc.tile_pool(name="ps_a", bufs=2,
                                              space="PSUM"))
        pso_p = st.enter_context(tc.tile_pool(name="ps_o", bufs=2,
                                              space="PSUM"))

        def Ablk(j, h, pe_bcast=None):
            emit_attnv(j, h, e2t.pop((j, h)), psa_p, pe_bcast=pe_bcast)

        Ablk(0, 0)
        Sblk(1, 2)
        Ablk(0, 1)
        Sblk(1, 3)
        Ablk(0, 2)
        Sblk(2, 0)
        Ablk(0, 3)
        emit_oproj(0, pso_p, chunks=[0])
        Sblk(2, 1)
        emit_oproj(0, pso_p, chunks=[1])
        Ablk(1, 0)
        emit_oproj(0, pso_p, chunks=[2])
        Sblk(2, 2)
        emit_oproj(0, pso_p, chunks=[3])
        Ablk(1, 1)
        Sblk(2, 3)
        Ablk(1, 2)
        Sblk(3, 0)
        Ablk(1, 3)
        emit_oproj(1, pso_p, chunks=[0])
        Sblk(3, 1)
        emit_oproj(1, pso_p, chunks=[1])
        Ablk(2, 0)
        emit_oproj(1, pso_p, chunks=[2])
        Ablk(2, 1)
        emit_oproj(1, pso_p, chunks=[3])
        Sblk(3, 2)
        Ablk(2, 2)
        Ablk(2, 3)
        emit_oproj(2, pso_p, chunks=[0, 1])
        Ablk(3, 0)
        emit_oproj(2, pso_p, chunks=[2, 3])
        Sblk(3, 3)
        Ablk(3, 1)
        Ablk(3, 2)
        Ablk(3, 3, pe_bcast=pso_p)
        emit_oproj(3, pso_p, act_copy=True, alt_pool=psa_p)

    nc.compile()
    return nc


_PROGRAM_CACHE = {}


def _get_program(seq=S):
    if seq not in _PROGRAM_CACHE:
        _PROGRAM_CACHE[seq] = build_program(seq)
    return _PROGRAM_CACHE[seq]


def make_in_maps(queries, keys, values, Wq, bq, Wk, bk, Wv, bv, Wo, bo):
    """Per-core input dicts implementing the sharding (bf16 on device)."""
    f32 = np.float32
    bf16 = ml_dtypes.bfloat16
    seq = np.asarray(queries).shape[1]
    xT = {}
    for b in range(B):
        xT[b] = tuple(
            np.ascontiguousarray(
                np.asarray(a[b], dtype=f32).T.astype(bf16))
            for a in (queries, keys, values)
        )
    Wq, Wk, Wv, Wo = (np.asarray(a, dtype=f32) for a in (Wq, Wk, Wv, Wo))
    bq, bk, bv = (np.asarray(a, dtype=f32) for a in (bq, bk, bv))
    in_maps = []
    for c in range(NCORES):
        b, g = divmod(c, GROUPS)
        cs = slice(g * CPC, (g + 1) * CPC)
        qT, kT, vT = xT[b]
        in_maps.append({
            "xqT": qT, "xkT": kT, "xvT": vT,
            "wq": np.ascontiguousarray(Wq[:, cs].astype(bf16)),
            "wk": np.ascontiguousarray(Wk[:, cs].astype(bf16)),
            "wv": np.ascontiguousarray(Wv[:, cs].astype(bf16)),
            "wo": np.ascontiguousarray(Wo[cs, :].astype(bf16)),
            "bq": np.ascontiguousarray(bq[cs].astype(bf16))[None, :],
            "bk": np.ascontiguousarray(bk[cs].astype(bf16))[None, :],
            "bv": np.ascontiguousarray(bv[cs].astype(bf16))[None, :],
            "ones_row": np.ones((1, QB), dtype=bf16),
            "ones_fr": np.ones((1, DK), dtype=f32),
            "vones": np.ones((P, (seq // P) * HPC), dtype=bf16),
        })
    return in_maps


def combine_outputs(results, bo):
    """Host all-reduce of the Wo row-shard partials + bias."""
    bo = np.asarray(bo, dtype=np.float32)
    outs = []
    for b in range(B):
        acc = results[b * GROUPS]["out"].astype(np.float32).copy()
        for g in range(1, GROUPS):
            acc += results[b * GROUPS + g]["out"]
        outs.append(acc + bo)
    return np.stack(outs)


def kernel(queries, keys, values, Wq, bq, Wk, bk, Wv, bv, Wo, bo):
    nc = _get_program()
    in_maps = make_in_maps(queries, keys, values, Wq, bq, Wk, bk, Wv, bv,
                           Wo, bo)
    res = run_bass_kernel_spmd(nc, in_maps, list(range(NCORES)))
    return combine_outputs(res.results, bo)


# revision 50
# speedup vs baseline: 1.0020x; 1.0020x over previous
"""MultiHeadAttention forward on 8 Trainium2 NeuronCores.

Sharding (Megatron-style tensor parallel x data parallel):
  core c (0..7): batch b = c // 4, head group g = c % 4 (4 of 16 heads).
  Wq/Wk/Wv column-sharded ([1024, 256] per core), Wo row-sharded
  ([256, 1024] per core). Each core computes a partial output
  [S, D] = attn(heads g) @ Wo_rows; the host sums the 4 partials per
  batch and adds bo (the "all-reduce" runs on host since full outputs
  are gathered anyway).

Projections/attnV/O run in bf16 (inputs converted on host; f32 PSUM
accumulate), halving HBM traffic vs f32r at the same PE rate. The
scores matmul runs in fp8-e4m3 DoubleRow perf mode (2x PE rate): K^T
and Q^T are drained from their projection psums into a [32, 2, s]
layout (d = 32*i + p) so each head's QK^T contracts as two 32-row
halves summed in the PE. Measured end-to-end rel err ~7.8e-3 vs the
2e-2 gate.

Schedule notes (the three serial chains that matter):
  - ACT runs the 128 softmax exps (~133us serial) plus a few psum
    drains placed in its natural stalls; it is kept fed from ~22us on.
  - Projection psums add biases via ones-row matmuls INSIDE the psum
    accumulation so the psum->SBUF drains are pure copies with no DMA
    dependency (the conservative DMA-queue semaphore encoding would
    otherwise stall each drain on every earlier DMA on its queue).
  - PE is warmed up on junk matmuls during the first DMA so Kb0 runs
    at full clock; K streams block-major so the first scores chunk
    only needs K-block 0 + Q-block 0.
  - attnV trails scores by ~4 blocks (e2 pool bufs=6); softmax
    normalization = DVE reciprocal + GPSIMD partition-broadcast (Pool
    engine, otherwise idle) + one DVE multiply.
  - O projection drains via DVE mid-kernel and via ACT for the last
    q-block (ACT is idle once the exps finish); host sums the 4
    row-shard partials per batch and adds bo.
"""

import math
from contextlib import ExitStack

import numpy as np
import ml_dtypes

import concourse.bass as bass
import concourse.mybir as mybir
import concourse.tile as tile
from concourse import bacc
from concourse.bass_utils import run_bass_kernel_spmd

P = 128
B, S, D, H = 2, 2048, 1024, 16
NCORES = 8
GROUPS = NCORES // B          # 4 head-groups
HPC = H // GROUPS             # 4 heads per core
DK = D // H                   # 64
CPC = HPC * DK                # 256 cols per core
NP = CPC // P                 # 2 head pairs per core
DC = D // P                   # 8 contraction chunks over D
QB = 512                      # q block (matmul moving free dim)

F32 = mybir.dt.float32
BF = mybir.dt.bfloat16
F8 = mybir.dt.float8e4


def build_program(seq=S):
    KT = seq // P             # k tiles
    NJ = seq // QB            # q blocks
    K2 = KT // 2              # two score k-tiles share one psum / exp op
    KT2 = KT // 2             # k-tile pairs for the V projection
    inv_sqrt_s = 1.0 / math.sqrt(S)  # reference scales by sqrt(full S)

    nc = bacc.Bacc("TRN2", target_bir_lowering=False, debug=False,
                   num_devices=NCORES)
    xqT = nc.declare_dram_parameter("xqT", [D, seq], BF, isOutput=False)
    xkT = nc.declare_dram_parameter("xkT", [D, seq], BF, isOutput=False)
    xvT = nc.declare_dram_parameter("xvT", [D, seq], BF, isOutput=False)
    wq = nc.declare_dram_parameter("wq", [D, CPC], BF, isOutput=False)
    wk = nc.declare_dram_parameter("wk", [D, CPC], BF, isOutput=False)
    wv = nc.declare_dram_parameter("wv", [D, CPC], BF, isOutput=False)
    wo = nc.declare_dram_parameter("wo", [CPC, D], BF, isOutput=False)
    bq = nc.declare_dram_parameter("bq", [1, CPC], BF, isOutput=False)
    bk = nc.declare_dram_parameter("bk", [1, CPC], BF, isOutput=False)
    bv = nc.declare_dram_parameter("bv", [1, CPC], BF, isOutput=False)
    ones_row = nc.declare_dram_parameter("ones_row", [1, QB], BF,
                                         isOutput=False)
    ones_fr = nc.declare_dram_parameter("ones_fr", [1, DK],
                                        mybir.dt.float32r, isOutput=False)
    vones = nc.declare_dram_parameter("vones", [P, KT * HPC], BF,
                                      isOutput=False)
    out = nc.declare_dram_parameter("out", [seq, D], BF, isOutput=True)

    xqT_r = xqT.rearrange("(dc p) s -> p dc s", p=P)
    xkT_r = xkT.rearrange("(dc p) s -> p dc s", p=P)
    xvT_r = xvT.rearrange("(dc p) s -> p dc s", p=P)

    with tile.TileContext(nc) as tc, ExitStack() as st:
        consts = st.enter_context(tc.tile_pool(name="consts", bufs=1))
        bq_sb = consts.tile([1, CPC], BF)
        bk_sb = consts.tile([1, CPC], BF)
        bv_sb = consts.tile([1, CPC], BF)
        ones_sb = consts.tile([1, QB], BF)
        ones_fr_sb = consts.tile([1, DK], mybir.dt.float32r)

        # Persistent activations. K^T/Q^T live in fp8 with the
        # DoubleRow layout: head h on partitions 32h..32h+31, free dims
        # (i, s) where d = 32 i + p — so the scores matmul runs in fp8
        # DoubleRow perf mode at 0.5 cycles/row (2x PE rate).
        # matmul operands need base partition in {0, 32, 64}: heads
        # 0-2 share tile 0 at bases 0/32/64, head 3 gets tile 1 base 0.
        kt_f8 = [consts.tile([P, 2, seq], F8, name=f"kt_f8_{t}")
                 for t in range(2)]
        qt_f8 = [[consts.tile([P, 2, QB], F8, name=f"qt_f8_{j}_{t}")
                  for t in range(2)] for j in range(NJ)]

        def hrow(h):
            t, b = (0, 32 * h) if h < 3 else (1, 0)
            return t, slice(b, b + 32)
        v_sb = consts.tile([P, KT, HPC, DK + 1], BF)
        at_j = [consts.tile([P, NP, QB], BF, name=f"at_j{j}")
                for j in range(NJ)]
        wo_sb = consts.tile([P, NP, D], BF)

        # Warm-up exp so the activation-table load happens during the
        # initial DMA instead of right before the first scores exp.
        warm = consts.tile([1, 1], F32)
        # biases are added inside the psum accumulation via a ones-row
        # matmul (like V) so the psum drains carry NO DMA dependency:
        # the conservative DMA-queue semaphore encoding would otherwise
        # stall each drain on every earlier-enqueued DMA on that queue.

        wqp = st.enter_context(tc.tile_pool(name="wqp", bufs=1))
        xqp = st.enter_context(tc.tile_pool(name="xqp", bufs=2))
        wq_sb = wqp.tile([P, DC, CPC], BF)
        wq_r = wq.rearrange("(dc p) c -> p dc c", p=P)

        def emit_qblock(j, pool, defer_drains=False, split_dma=False):
            xt = xqp.tile([P, DC, QB], BF, tag="xq")
            qsl = xqT_r[:, :, j * QB:(j + 1) * QB]
            if split_dma:
                # halve the first block's DMA so dc 0-3 matmuls start a
                # transfer earlier during the serial bootstrap
                nc.sync.dma_start(xt[:, 0:DC // 2], qsl[:, 0:DC // 2])
                nc.sync.dma_start(xt[:, DC // 2:], qsl[:, DC // 2:])
            else:
                nc.sync.dma_start(xt[:], qsl)
            ps = [pool.tile([P, QB], F32, tag="k", name=f"psq_{j}_{pi}")
                  for pi in range(NP)]
            for dc in range(DC):
                for pi in range(NP):
                    nc.tensor.matmul(
                        ps[pi][:],
                        wq_sb[:, dc, pi * P:(pi + 1) * P],
                        xt[:, dc],
                        start=(dc == 0), stop=False,
                    )
            for pi in range(NP):
                nc.tensor.matmul(  # += bq^T @ ones  (bias add)
                    ps[pi][:], bq_sb[:, pi * P:(pi + 1) * P], ones_sb[:],
                    start=False, stop=True,
                )

            def drains(pis):
                for pi in pis:
                    for hp in range(2):
                        for i in range(2):
                            r = hp * 64 + 32 * i
                            t, rows = hrow(2 * pi + hp)
                            nc.vector.tensor_copy(
                                qt_f8[j][t][rows, i, :],
                                ps[pi][r:r + 32, :])
            if defer_drains == "p1":
                drains([0])
                return lambda: drains([1])
            if defer_drains:
                return lambda: drains(range(NP))
            drains(range(NP))

        # K projection, block-major like Q (one [P, DC, QB] DMA per
        # block) so kt_p columns drain progressively and the first
        # scores exps can start right after K's matmuls.
        xkp = st.enter_context(tc.tile_pool(name="xkp", bufs=2))
        wkp = st.enter_context(tc.tile_pool(name="wkp", bufs=1))
        wk_sb = wkp.tile([P, DC, CPC], BF)
        wk_r = wk.rearrange("(dc p) c -> p dc c", p=P)

        def emit_kblock(qc, pool, act_p1=False, split_dma=False,
                        defer_p0h1=False):
            xt = xkp.tile([P, DC, QB], BF, tag="xk")
            ksl = xkT_r[:, :, qc * QB:(qc + 1) * QB]
            if split_dma:
                nc.sync.dma_start(xt[:, 0:DC // 2], ksl[:, 0:DC // 2])
                nc.sync.dma_start(xt[:, DC // 2:], ksl[:, DC // 2:])
            else:
                nc.sync.dma_start(xt[:], ksl)
            ps = [pool.tile([P, QB], F32, tag="k", name=f"psk_{qc}_{pi}")
                  for pi in range(NP)]
            for dc in range(DC):
                for pi in range(NP):
                    nc.tensor.matmul(
                        ps[pi][:],
                        wk_sb[:, dc, pi * P:(pi + 1) * P],
                        xt[:, dc],
                        start=(dc == 0), stop=False,
                    )
            for pi in range(NP):
                nc.tensor.matmul(  # += bk^T @ ones  (bias add)
                    ps[pi][:], bk_sb[:, pi * P:(pi + 1) * P], ones_sb[:],
                    start=False, stop=True,
                )
            def kdrain(pi, hp):
                for i in range(2):
                    r = hp * 64 + 32 * i
                    t, rows = hrow(2 * pi + hp)
                    dst = kt_f8[t][rows, i, qc * QB:(qc + 1) * QB]
                    if pi == 1 and act_p1:
                        # heads 2/3 drain on ACT: they are needed a
                        # whole exp-block later and fill the early
                        # ACT stalls, lightening the serial DVE queue
                        nc.scalar.copy(dst, ps[pi][r:r + 32, :])
                    else:
                        nc.vector.tensor_copy(dst, ps[pi][r:r + 32, :])
            kdrain(0, 0)
            if not defer_p0h1:
                kdrain(0, 1)
            kdrain(1, 0)
            kdrain(1, 1)
            if defer_p0h1:
                return lambda: kdrain(0, 1)

        # ---- attention pipeline pieces ----
        ep = st.enter_context(tc.tile_pool(name="epool", bufs=6))
        rp = st.enter_context(tc.tile_pool(name="rpool", bufs=2))
        op = st.enter_context(tc.tile_pool(name="opool", bufs=8))

        def emit_scores(j, h, e2, k2s, pss_p):
            t, rows = hrow(h)
            for k2 in k2s:
                pss = pss_p.tile([P, 2 * QB], F32, tag="s",
                                 name=f"pss_{j}_{h}_{k2}")
                for half in range(2):
                    kt = 2 * k2 + half
                    nc.tensor.matmul(
                        pss[:, half * QB:(half + 1) * QB],
                        kt_f8[t][rows, :, kt * P:(kt + 1) * P],
                        qt_f8[j][t][rows, :, :],
                        start=True, stop=True,
                        perf_mode=mybir.MatmulPerfMode.DoubleRow,
                    )
                nc.scalar.activation(
                    e2[:, k2], pss[:],
                    mybir.ActivationFunctionType.Exp,
                    scale=inv_sqrt_s,
                )

        e2t = {}

        def Sblk(j, h, k2s=None, pss_pool=None):
            if k2s is None or k2s[0] == 0:
                e2 = ep.tile([P, K2, 2 * QB], BF, tag="E",
                             name=f"e2_{j}_{h}")
                e2t[(j, h)] = e2
            emit_scores(j, h, e2t[(j, h)], k2s or range(K2),
                        pss_pool or pss_p)

        # ---- V projection (kt-pair-major, one psum bank per pair) ----
        v_stack = ExitStack()
        q_stack = ExitStack()

        def emit_vblock(kt2, xvp, psv_p):
            xt = xvp.tile([P, DC, 2 * P], BF, tag="xv")
            nc.sync.dma_start(
                xt[:], xvT_r[:, :, kt2 * 2 * P:(kt2 + 1) * 2 * P])
            psv = psv_p.tile([P, 2, CPC], F32, tag="v", name=f"psv_{kt2}")
            for dc in range(DC):
                for half in range(2):
                    nc.tensor.matmul(
                        psv[:, half],
                        xt[:, dc, half * P:(half + 1) * P],
                        wv_sb[:, dc],
                        start=(dc == 0 and half == 0), stop=False,
                    )
            for half in range(2):
                nc.tensor.matmul(  # += ones^T @ bv  (bias add)
                    psv[:, half], ones_sb[:, :P], bv_sb[:],
                    start=False, stop=(half == 1),
                )
            for half in range(2):
                nc.vector.tensor_copy(
                    v_sb[:, 2 * kt2 + half, :, 0:DK],
                    psv[:, half].rearrange("p (h d) -> p h d", h=HPC),
                )

        def emit_attnv(j, h, e2, psa_p, pe_bcast=None):
            hp, hj = h % 2, h // 2
            prow = slice(hp * DK, (hp + 1) * DK)
            psa = psa_p.tile([P, QB], F32, tag="a", name=f"psa_{j}_{h}")
            for kt in range(KT):
                nc.tensor.matmul(
                    psa[:DK + 1],
                    v_sb[:, kt, h, :],
                    e2[:, kt // 2, (kt % 2) * QB:(kt % 2 + 1) * QB],
                    start=(kt == 0), stop=(kt == KT - 1),
                )
            # softmax denominator is psa row DK; normalize via DVE recip +
            # GPSIMD partition-broadcast (Pool engine) + DVE multiply.
            # On the last block the Pool round-trip is on the critical
            # tail: broadcast via a PE matmul (f32r, exact) into a
            # borrowed psum bank instead.
            if pe_bcast is not None:
                rc = rp.tile([1, QB], mybir.dt.float32r, tag="rcr",
                             bufs=1)
                with nc.allow_low_precision(
                        reason="f32r reciprocal for matmul bcast"):
                    nc.vector.reciprocal(rc[:], psa[DK:DK + 1, :])
                prc = pe_bcast.tile([P, QB], F32, tag="o",
                                    name=f"prc_{j}_{h}")
                nc.tensor.matmul(prc[:DK], ones_fr_sb[:], rc[:],
                                 start=True, stop=True)
                # tensor_tensor may read only one PSUM operand: stage
                # psa through SBUF (overlaps the PE broadcast matmul)
                atmp = rp.tile([DK, QB], F32, tag="prc")
                nc.vector.tensor_copy(atmp[:], psa[:DK])
                nc.vector.tensor_tensor(
                    at_j[j][prow, hj, :], atmp[:], prc[:DK],
                    mybir.AluOpType.mult,
                )
                return
            rc = rp.tile([1, QB], F32, tag="rc")
            nc.vector.reciprocal(rc[:], psa[DK:DK + 1, :])
            prc = rp.tile([DK, QB], F32, tag="prc")
            nc.gpsimd.partition_broadcast(prc[:], rc[:])
            nc.vector.tensor_tensor(
                at_j[j][prow, hj, :], psa[:DK], prc[:],
                mybir.AluOpType.mult,
            )

        def emit_oproj(j, pso_p, act_copy=False, chunks=None,
                       alt_pool=None):
            for ql in (range(QB // P) if chunks is None else chunks):
                qt0 = j * (QB // P) + ql
                for nh in range(D // QB):
                    o_sb = op.tile([P, QB], BF, tag="o_sb",
                                   name=f"osb_{qt0}_{nh}")
                    # tail block: borrow the attnV psum banks (drained
                    # by then) so four banks rotate instead of two
                    pp = alt_pool if (alt_pool is not None
                                      and (ql * 2 + nh) % 2 == 1) else pso_p
                    pso = pp.tile([P, QB], F32,
                                  tag="a" if pp is alt_pool else "o",
                                  name=f"pso_{qt0}_{nh}")
                    for dj in range(NP):
                        nc.tensor.matmul(
                            pso[:],
                            at_j[j][:, dj, ql * P:(ql + 1) * P],
                            wo_sb[:, dj, nh * QB:(nh + 1) * QB],
                            start=(dj == 0), stop=(dj == NP - 1),
                        )
                    if act_copy and (ql * 2 + nh) % 2 == 0:
                        # tail block: alternate ACT/DVE copies (both are
                        # idle once the exps and norms finish)
                        nc.scalar.copy(o_sb[:], pso[:])
                    else:
                        nc.vector.tensor_copy(o_sb[:], pso[:])
                    nc.sync.dma_start(
                        out[qt0 * P:(qt0 + 1) * P,
                            nh * QB:(nh + 1) * QB],
                        o_sb[:],
                    )

        # ---- interleaved emission schedule ----
        # PE queue order == execution order. Scores blocks are
        # ACT-throttled (~8.3us each via the pss double-buffer), so the
        # Q/K/V projection matmuls placed between them execute when
        # their DMA lands, filling PE gaps. attnV trails scores by 3-4
        # blocks (e2 pool bufs=5).
        pss_p = st.enter_context(tc.tile_pool(name="ps_s", bufs=2,
                                              space="PSUM"))
        with tc.tile_pool(name="ps_kq", bufs=4, space="PSUM") as ps_k:
            nc.sync.dma_start(wk_sb[:], wk_r)
            # PE warm-up: ramp the tensor engine to full clock on junk
            # matmuls over wk while xkb0 streams, and prefetch the exp
            # activation table, so Kb0 runs at full rate immediately.
            nc.sync.dma_start(bk_sb[:], bk[:])
            nc.sync.dma_start(ones_sb[:], ones_row[:])
            wps = pss_p.tile([P, 2 * QB], F32, tag="s", name="warm_ps")
            for w in range(8):
                nc.tensor.matmul(wps[:, :QB], wk_sb[:, 0, :P],
                                 wk_sb[:, 0:2, :], start=True, stop=True)
            nc.scalar.activation(warm[:], wk_sb[0:1, 0:1, 0:1],
                                 mybir.ActivationFunctionType.Exp)
            emit_kblock(0, ps_k, act_p1=True)
            nc.sync.dma_start(wq_sb[:], wq_r)
            nc.sync.dma_start(bq_sb[:], bq[:])
            q0_p1 = emit_qblock(0, ps_k, defer_drains="p1")
            Sblk(0, 0, range(0, 2))
            emit_kblock(1, ps_k, act_p1=True)
            q0_p1()
            Sblk(0, 0, range(2, 4))
            k2_h1 = emit_kblock(2, ps_k, act_p1=True, defer_p0h1=True)
            Sblk(0, 0, range(4, 6))
            k3_h1 = emit_kblock(3, ps_k, act_p1=True, defer_p0h1=True)
            Sblk(0, 0, range(6, 8))
            emit_qblock(1, ps_k)
            k2_h1()
            k3_h1()
        q_psum = q_stack.enter_context(
            tc.tile_pool(name="ps_q", bufs=2, space="PSUM"))
        Sblk(0, 1)
        emit_qblock(2, q_psum)
        Sblk(0, 2)
        q3_drains = emit_qblock(3, q_psum, defer_drains=True)
        xvp = v_stack.enter_context(tc.tile_pool(name="xvp", bufs=2))
        wvp = v_stack.enter_context(tc.tile_pool(name="wvp", bufs=1))
        psv_p = v_stack.enter_context(
            tc.tile_pool(name="ps_v", bufs=2, space="PSUM"))
        wv_sb = wvp.tile([P, DC, CPC], BF)
        nc.sync.dma_start(wv_sb[:], wv.rearrange("(dc p) c -> p dc c", p=P))
        nc.sync.dma_start(bv_sb[:], bv[:])
        nc.sync.dma_start(ones_sb[:], ones_row[:])
        emit_vblock(0, xvp, psv_p)
        emit_vblock(1, xvp, psv_p)
        Sblk(0, 3)
        emit_vblock(2, xvp, psv_p)
        emit_vblock(3, xvp, psv_p)
        Sblk(1, 0)
        emit_vblock(4, xvp, psv_p)
        emit_vblock(5, xvp, psv_p)
        Sblk(1, 1)
        emit_vblock(6, xvp, psv_p)
        emit_vblock(7, xvp, psv_p)
        with nc.allow_non_contiguous_dma(
                reason="one-time 16KB ones-column init"):
            nc.sync.dma_start(
                v_sb[:, :, :, DK:DK + 1],
                vones.rearrange("p (kt h) -> p kt h",
                                kt=KT, h=HPC)[:, :, :, None],
            )
        nc.sync.dma_start(wo_sb[:],
                          wo.rearrange("(dj p) n -> p dj n", p=P))
        nc.sync.dma_start(ones_fr_sb[:], ones_fr[:])
        q3_drains()
        v_stack.close()
        q_stack.close()
        psa_p = st.enter_context(tc.tile_pool(name="ps_a", bufs=2,
                                              space="PSUM"))
        pso_p = st.enter_context(tc.tile_pool(name="ps_o", bufs=2,
                                              space="PSUM"))

        def Ablk(j, h, pe_bcast=None):
            emit_attnv(j, h, e2t.pop((j, h)), psa_p, pe_bcast=pe_bcast)

        Ablk(0, 0)
        Sblk(1, 2)
        Ablk(0, 1)
        Sblk(1, 3)
        Ablk(0, 2)
        Sblk(2, 0)
        Ablk(0, 3)
        emit_oproj(0, pso_p, chunks=[0])
        Sblk(2, 1)
        emit_oproj(0, pso_p, chunks=[1])
        Ablk(1, 0)
        emit_oproj(0, pso_p, chunks=[2])
        Sblk(2, 2)
        emit_oproj(0, pso_p, chunks=[3])
        Ablk(1, 1)
        Sblk(2, 3)
        Ablk(1, 2)
        Sblk(3, 0)
        Ablk(1, 3)
        emit_oproj(1, pso_p, chunks=[0])
        Sblk(3, 1)
        emit_oproj(1, pso_p, chunks=[1])
        Ablk(2, 0)
        emit_oproj(1, pso_p, chunks=[2])
        Ablk(2, 1)
        emit_oproj(1, pso_p, chunks=[3])
        Sblk(3, 2)
        Ablk(2, 2)
        Ablk(2, 3)
        emit_oproj(2, pso_p, chunks=[0, 1])
        Ablk(3, 0)
        emit_oproj(2, pso_p, chunks=[2, 3])
        Sblk(3, 3)
        Ablk(3, 1)
        Ablk(3, 2)
        Ablk(3, 3, pe_bcast=pso_p)
        emit_oproj(3, pso_p, act_copy=True, alt_pool=psa_p)

    nc.compile()
    return nc


_PROGRAM_CACHE = {}


def _get_program(seq=S):
    if seq not in _PROGRAM_CACHE:
        _PROGRAM_CACHE[seq] = build_program(seq)
    return _PROGRAM_CACHE[seq]


def make_in_maps(queries, keys, values, Wq, bq, Wk, bk, Wv, bv, Wo, bo):
    """Per-core input dicts implementing the sharding (bf16 on device)."""
    f32 = np.float32
    bf16 = ml_dtypes.bfloat16
    seq = np.asarray(queries).shape[1]
    xT = {}
    for b in range(B):
        xT[b] = tuple(
            np.ascontiguousarray(
                np.asarray(a[b], dtype=f32).T.astype(bf16))
            for a in (queries, keys, values)
        )
    Wq, Wk, Wv, Wo = (np.asarray(a, dtype=f32) for a in (Wq, Wk, Wv, Wo))
    bq, bk, bv = (np.asarray(a, dtype=f32) for a in (bq, bk, bv))
    in_maps = []
    for c in range(NCORES):
        b, g = divmod(c, GROUPS)
        cs = slice(g * CPC, (g + 1) * CPC)
        qT, kT, vT = xT[b]
        in_maps.append({
            "xqT": qT, "xkT": kT, "xvT": vT,
            "wq": np.ascontiguousarray(Wq[:, cs].astype(bf16)),
            "wk": np.ascontiguousarray(Wk[:, cs].astype(bf16)),
            "wv": np.ascontiguousarray(Wv[:, cs].astype(bf16)),
            "wo": np.ascontiguousarray(Wo[cs, :].astype(bf16)),
            "bq": np.ascontiguousarray(bq[cs].astype(bf16))[None, :],
            "bk": np.ascontiguousarray(bk[cs].astype(bf16))[None, :],
            "bv": np.ascontiguousarray(bv[cs].astype(bf16))[None, :],
            "ones_row": np.ones((1, QB), dtype=bf16),
            "ones_fr": np.ones((1, DK), dtype=f32),
            "vones": np.ones((P, (seq // P) * HPC), dtype=bf16),
        })
    return in_maps


def combine_outputs(results, bo):
    """Host all-reduce of the Wo row-shard partials + bias."""
    bo = np.asarray(bo, dtype=np.float32)
    outs = []
    for b in range(B):
        acc = results[b * GROUPS]["out"].astype(np.float32).copy()
        for g in range(1, GROUPS):
            acc += results[b * GROUPS + g]["out"]
        outs.append(acc + bo)
    return np.stack(outs)


def kernel(queries, keys, values, Wq, bq, Wk, bk, Wv, bv, Wo, bo):
    nc = _get_program()
    in_maps = make_in_maps(queries, keys, values, Wq, bq, Wk, bk, Wv, bv,
                           Wo, bo)
    res = run_bass_kernel_spmd(nc, in_maps, list(range(NCORES)))
    return combine_outputs(res.results, bo)


# revision 51
# speedup vs baseline: 1.0142x; 1.0122x over previous
"""MultiHeadAttention forward on 8 Trainium2 NeuronCores.

Sharding (Megatron-style tensor parallel x data parallel):
  core c (0..7): batch b = c // 4, head group g = c % 4 (4 of 16 heads).
  Wq/Wk/Wv column-sharded ([1024, 256] per core), Wo row-sharded
  ([256, 1024] per core). Each core computes a partial output
  [S, D] = attn(heads g) @ Wo_rows; the host sums the 4 partials per
  batch and adds bo (the "all-reduce" runs on host since full outputs
  are gathered anyway).

Projections/attnV/O run in bf16 (inputs converted on host; f32 PSUM
accumulate), halving HBM traffic vs f32r at the same PE rate. The
scores matmul runs in fp8-e4m3 DoubleRow perf mode (2x PE rate): K^T
and Q^T are drained from their projection psums into a [32, 2, s]
layout (d = 32*i + p) so each head's QK^T contracts as two 32-row
halves summed in the PE. Measured end-to-end rel err ~7.8e-3 vs the
2e-2 gate.

Schedule notes (the three serial chains that matter):
  - ACT runs the 128 softmax exps (~133us serial) plus a few psum
    drains placed in its natural stalls; it is kept fed from ~22us on.
  - Projection psums add biases via ones-row matmuls INSIDE the psum
    accumulation so the psum->SBUF drains are pure copies with no DMA
    dependency (the conservative DMA-queue semaphore encoding would
    otherwise stall each drain on every earlier DMA on its queue).
  - PE is warmed up on junk matmuls during the first DMA so Kb0 runs
    at full clock; K streams block-major so the first scores chunk
    only needs K-block 0 + Q-block 0.
  - attnV trails scores by ~4 blocks (e2 pool bufs=6); softmax
    normalization = DVE reciprocal + GPSIMD partition-broadcast (Pool
    engine, otherwise idle) + one DVE multiply.
  - O projection drains via DVE mid-kernel and via ACT for the last
    q-block (ACT is idle once the exps finish); host sums the 4
    row-shard partials per batch and adds bo.
"""

import math
from contextlib import ExitStack

import numpy as np
import ml_dtypes

import concourse.bass as bass
import concourse.mybir as mybir
import concourse.tile as tile
from concourse import bacc
from concourse.bass_utils import run_bass_kernel_spmd

P = 128
B, S, D, H = 2, 2048, 1024, 16
NCORES = 8
GROUPS = NCORES // B          # 4 head-groups
HPC = H // GROUPS             # 4 heads per core
DK = D // H                   # 64
CPC = HPC * DK                # 256 cols per core
NP = CPC // P                 # 2 head pairs per core
DC = D // P                   # 8 contraction chunks over D
QB = 512                      # q block (matmul moving free dim)

F32 = mybir.dt.float32
BF = mybir.dt.bfloat16
F8 = mybir.dt.float8e4


def build_program(seq=S):
    KT = seq // P             # k tiles
    NJ = seq // QB            # q blocks
    K2 = KT // 2              # two score k-tiles share one psum / exp op
    KT2 = KT // 2             # k-tile pairs for the V projection
    inv_sqrt_s = 1.0 / math.sqrt(S)  # reference scales by sqrt(full S)

    nc = bacc.Bacc("TRN2", target_bir_lowering=False, debug=False,
                   num_devices=NCORES)
    xqT = nc.declare_dram_parameter("xqT", [D, seq], BF, isOutput=False)
    xkT = nc.declare_dram_parameter("xkT", [D, seq], BF, isOutput=False)
    xvT = nc.declare_dram_parameter("xvT", [D, seq], BF, isOutput=False)
    wq = nc.declare_dram_parameter("wq", [D, CPC], BF, isOutput=False)
    wk = nc.declare_dram_parameter("wk", [D, CPC], BF, isOutput=False)
    wv = nc.declare_dram_parameter("wv", [D, CPC], BF, isOutput=False)
    wo = nc.declare_dram_parameter("wo", [CPC, D], BF, isOutput=False)
    bq = nc.declare_dram_parameter("bq", [1, CPC], BF, isOutput=False)
    bk = nc.declare_dram_parameter("bk", [1, CPC], BF, isOutput=False)
    bv = nc.declare_dram_parameter("bv", [1, CPC], BF, isOutput=False)
    ones_row = nc.declare_dram_parameter("ones_row", [1, QB], BF,
                                         isOutput=False)
    ones_fr = nc.declare_dram_parameter("ones_fr", [1, DK],
                                        mybir.dt.float32r, isOutput=False)
    vones = nc.declare_dram_parameter("vones", [P, KT * HPC], BF,
                                      isOutput=False)
    out = nc.declare_dram_parameter("out", [seq, D], BF, isOutput=True)

    xqT_r = xqT.rearrange("(dc p) s -> p dc s", p=P)
    xkT_r = xkT.rearrange("(dc p) s -> p dc s", p=P)
    xvT_r = xvT.rearrange("(dc p) s -> p dc s", p=P)

    with tile.TileContext(nc) as tc, ExitStack() as st:
        consts = st.enter_context(tc.tile_pool(name="consts", bufs=1))
        bq_sb = consts.tile([1, CPC], BF)
        bk_sb = consts.tile([1, CPC], BF)
        bv_sb = consts.tile([1, CPC], BF)
        ones_sb = consts.tile([1, QB], BF)
        ones_fr_sb = consts.tile([1, DK], mybir.dt.float32r)

        # Persistent activations. K^T/Q^T live in fp8 with the
        # DoubleRow layout: head h on partitions 32h..32h+31, free dims
        # (i, s) where d = 32 i + p — so the scores matmul runs in fp8
        # DoubleRow perf mode at 0.5 cycles/row (2x PE rate).
        # matmul operands need base partition in {0, 32, 64}: heads
        # 0-2 share tile 0 at bases 0/32/64, head 3 gets tile 1 base 0.
        kt_f8 = [consts.tile([P, 2, seq], F8, name=f"kt_f8_{t}")
                 for t in range(2)]
        qt_f8 = [[consts.tile([P, 2, QB], F8, name=f"qt_f8_{j}_{t}")
                  for t in range(2)] for j in range(NJ)]

        def hrow(h):
            t, b = (0, 32 * h) if h < 3 else (1, 0)
            return t, slice(b, b + 32)
        v_sb = consts.tile([P, KT, HPC, DK + 1], BF)
        at_j = [consts.tile([P, NP, QB], BF, name=f"at_j{j}")
                for j in range(NJ)]
        wo_sb = consts.tile([P, NP, D], BF)

        # Warm-up exp so the activation-table load happens during the
        # initial DMA instead of right before the first scores exp.
        warm = consts.tile([1, 1], F32)
        # biases are added inside the psum accumulation via a ones-row
        # matmul (like V) so the psum drains carry NO DMA dependency:
        # the conservative DMA-queue semaphore encoding would otherwise
        # stall each drain on every earlier-enqueued DMA on that queue.

        wqp = st.enter_context(tc.tile_pool(name="wqp", bufs=1))
        xqp = st.enter_context(tc.tile_pool(name="xqp", bufs=2))
        wq_sb = wqp.tile([P, DC, CPC], BF)
        wq_r = wq.rearrange("(dc p) c -> p dc c", p=P)

        def emit_qblock(j, pool, defer_drains=False, split_dma=False):
            xt = xqp.tile([P, DC, QB], BF, tag="xq")
            qsl = xqT_r[:, :, j * QB:(j + 1) * QB]
            if split_dma:
                # halve the first block's DMA so dc 0-3 matmuls start a
                # transfer earlier during the serial bootstrap
                nc.sync.dma_start(xt[:, 0:DC // 2], qsl[:, 0:DC // 2])
                nc.sync.dma_start(xt[:, DC // 2:], qsl[:, DC // 2:])
            else:
                nc.sync.dma_start(xt[:], qsl)
            ps = [pool.tile([P, QB], F32, tag="k", name=f"psq_{j}_{pi}")
                  for pi in range(NP)]
            for dc in range(DC):
                for pi in range(NP):
                    nc.tensor.matmul(
                        ps[pi][:],
                        wq_sb[:, dc, pi * P:(pi + 1) * P],
                        xt[:, dc],
                        start=(dc == 0), stop=False,
                    )
            for pi in range(NP):
                nc.tensor.matmul(  # += bq^T @ ones  (bias add)
                    ps[pi][:], bq_sb[:, pi * P:(pi + 1) * P], ones_sb[:],
                    start=False, stop=True,
                )

            def drains(pis):
                for pi in pis:
                    for hp in range(2):
                        for i in range(2):
                            r = hp * 64 + 32 * i
                            t, rows = hrow(2 * pi + hp)
                            nc.vector.tensor_copy(
                                qt_f8[j][t][rows, i, :],
                                ps[pi][r:r + 32, :])
            if defer_drains == "p1":
                drains([0])
                return lambda: drains([1])
            if defer_drains:
                return lambda: drains(range(NP))
            drains(range(NP))

        # K projection, block-major like Q (one [P, DC, QB] DMA per
        # block) so kt_p columns drain progressively and the first
        # scores exps can start right after K's matmuls.
        xkp = st.enter_context(tc.tile_pool(name="xkp", bufs=2))
        wkp = st.enter_context(tc.tile_pool(name="wkp", bufs=1))
        wk_sb = wkp.tile([P, DC, CPC], BF)
        wk_r = wk.rearrange("(dc p) c -> p dc c", p=P)

        def emit_kblock(qc, pool, act_p1=False, split_dma=False,
                        defer_p0h1=False):
            xt = xkp.tile([P, DC, QB], BF, tag="xk")
            ksl = xkT_r[:, :, qc * QB:(qc + 1) * QB]
            if split_dma:
                nc.sync.dma_start(xt[:, 0:DC // 2], ksl[:, 0:DC // 2])
                nc.sync.dma_start(xt[:, DC // 2:], ksl[:, DC // 2:])
            else:
                nc.sync.dma_start(xt[:], ksl)
            ps = [pool.tile([P, QB], F32, tag="k", name=f"psk_{qc}_{pi}")
                  for pi in range(NP)]
            for dc in range(DC):
                for pi in range(NP):
                    nc.tensor.matmul(
                        ps[pi][:],
                        wk_sb[:, dc, pi * P:(pi + 1) * P],
                        xt[:, dc],
                        start=(dc == 0), stop=False,
                    )
            for pi in range(NP):
                nc.tensor.matmul(  # += bk^T @ ones  (bias add)
                    ps[pi][:], bk_sb[:, pi * P:(pi + 1) * P], ones_sb[:],
                    start=False, stop=True,
                )
            def kdrain(pi, hp):
                for i in range(2):
                    r = hp * 64 + 32 * i
                    t, rows = hrow(2 * pi + hp)
                    dst = kt_f8[t][rows, i, qc * QB:(qc + 1) * QB]
                    if pi == 1 and act_p1:
                        # heads 2/3 drain on ACT: they are needed a
                        # whole exp-block later and fill the early
                        # ACT stalls, lightening the serial DVE queue
                        nc.scalar.copy(dst, ps[pi][r:r + 32, :])
                    else:
                        nc.vector.tensor_copy(dst, ps[pi][r:r + 32, :])
            kdrain(0, 0)
            if not defer_p0h1:
                kdrain(0, 1)
            kdrain(1, 0)
            kdrain(1, 1)
            if defer_p0h1:
                return lambda: kdrain(0, 1)

        # ---- attention pipeline pieces ----
        ep = st.enter_context(tc.tile_pool(name="epool", bufs=6))
        rp = st.enter_context(tc.tile_pool(name="rpool", bufs=2))
        op = st.enter_context(tc.tile_pool(name="opool", bufs=8))

        def emit_scores(j, h, e2, k2s, pss_p):
            t, rows = hrow(h)
            for k2 in k2s:
                pss = pss_p.tile([P, 2 * QB], F32, tag="s",
                                 name=f"pss_{j}_{h}_{k2}")
                for half in range(2):
                    kt = 2 * k2 + half
                    nc.tensor.matmul(
                        pss[:, half * QB:(half + 1) * QB],
                        kt_f8[t][rows, :, kt * P:(kt + 1) * P],
                        qt_f8[j][t][rows, :, :],
                        start=True, stop=True,
                        perf_mode=mybir.MatmulPerfMode.DoubleRow,
                    )
                nc.scalar.activation(
                    e2[:, k2], pss[:],
                    mybir.ActivationFunctionType.Exp,
                    scale=inv_sqrt_s,
                )

        e2t = {}

        def Sblk(j, h, k2s=None, pss_pool=None):
            if k2s is None or k2s[0] == 0:
                e2 = ep.tile([P, K2, 2 * QB], BF, tag="E",
                             name=f"e2_{j}_{h}")
                e2t[(j, h)] = e2
            emit_scores(j, h, e2t[(j, h)], k2s or range(K2),
                        pss_pool or pss_p)

        # ---- V projection (kt-pair-major, one psum bank per pair) ----
        v_stack = ExitStack()
        q_stack = ExitStack()

        def emit_vblock(kt2, xvp, psv_p):
            xt = xvp.tile([P, DC, 2 * P], BF, tag="xv")
            nc.sync.dma_start(
                xt[:], xvT_r[:, :, kt2 * 2 * P:(kt2 + 1) * 2 * P])
            psv = psv_p.tile([P, 2, CPC], F32, tag="v", name=f"psv_{kt2}")
            for dc in range(DC):
                for half in range(2):
                    nc.tensor.matmul(
                        psv[:, half],
                        xt[:, dc, half * P:(half + 1) * P],
                        wv_sb[:, dc],
                        start=(dc == 0 and half == 0), stop=False,
                    )
            for half in range(2):
                nc.tensor.matmul(  # += ones^T @ bv  (bias add)
                    psv[:, half], ones_sb[:, :P], bv_sb[:],
                    start=False, stop=(half == 1),
                )
            for half in range(2):
                nc.vector.tensor_copy(
                    v_sb[:, 2 * kt2 + half, :, 0:DK],
                    psv[:, half].rearrange("p (h d) -> p h d", h=HPC),
                )

        def emit_attnv(j, h, e2, psa_p, pe_bcast=None):
            hp, hj = h % 2, h // 2
            prow = slice(hp * DK, (hp + 1) * DK)
            psa = psa_p.tile([P, QB], F32, tag="a", name=f"psa_{j}_{h}")
            for kt in range(KT):
                nc.tensor.matmul(
                    psa[:DK + 1],
                    v_sb[:, kt, h, :],
                    e2[:, kt // 2, (kt % 2) * QB:(kt % 2 + 1) * QB],
                    start=(kt == 0), stop=(kt == KT - 1),
                )
            # softmax denominator is psa row DK; normalize via DVE recip +
            # GPSIMD partition-broadcast (Pool engine) + DVE multiply.
            # On the last block the Pool round-trip is on the critical
            # tail: broadcast via a PE matmul (f32r, exact) into a
            # borrowed psum bank instead.
            if pe_bcast is not None:
                rc = rp.tile([1, QB], mybir.dt.float32r, tag="rcr",
                             bufs=1)
                with nc.allow_low_precision(
                        reason="f32r reciprocal for matmul bcast"):
                    nc.vector.reciprocal(rc[:], psa[DK:DK + 1, :])
                prc = pe_bcast.tile([P, QB], F32, tag="o",
                                    name=f"prc_{j}_{h}")
                nc.tensor.matmul(prc[:DK], ones_fr_sb[:], rc[:],
                                 start=True, stop=True)
                # tensor_tensor may read only one PSUM operand: stage
                # psa through SBUF (overlaps the PE broadcast matmul)
                atmp = rp.tile([DK, QB], F32, tag="prc")
                nc.vector.tensor_copy(atmp[:], psa[:DK])
                nc.vector.tensor_tensor(
                    at_j[j][prow, hj, :], atmp[:], prc[:DK],
                    mybir.AluOpType.mult,
                )
                return
            rc = rp.tile([1, QB], F32, tag="rc")
            nc.vector.reciprocal(rc[:], psa[DK:DK + 1, :])
            prc = rp.tile([DK, QB], F32, tag="prc")
            nc.gpsimd.partition_broadcast(prc[:], rc[:])
            nc.vector.tensor_tensor(
                at_j[j][prow, hj, :], psa[:DK], prc[:],
                mybir.AluOpType.mult,
            )

        def emit_oproj(j, pso_p, act_copy=False, chunks=None,
                       alt_pool=None):
            for ql in (range(QB // P) if chunks is None else chunks):
                qt0 = j * (QB // P) + ql
                for nh in range(D // QB):
                    o_sb = op.tile([P, QB], BF, tag="o_sb",
                                   name=f"osb_{qt0}_{nh}")
                    # tail block: borrow the attnV psum banks (drained
                    # by then) so four banks rotate instead of two
                    pp = alt_pool if (alt_pool is not None
                                      and (ql * 2 + nh) % 2 == 1) else pso_p
                    pso = pp.tile([P, QB], F32,
                                  tag="a" if pp is alt_pool else "o",
                                  name=f"pso_{qt0}_{nh}")
                    for dj in range(NP):
                        nc.tensor.matmul(
                            pso[:],
                            at_j[j][:, dj, ql * P:(ql + 1) * P],
                            wo_sb[:, dj, nh * QB:(nh + 1) * QB],
                            start=(dj == 0), stop=(dj == NP - 1),
                        )
                    if act_copy and (ql * 2 + nh) % 2 == 0:
                        # tail block: alternate ACT/DVE copies (both are
                        # idle once the exps and norms finish)
                        nc.scalar.copy(o_sb[:], pso[:])
                    else:
                        nc.vector.tensor_copy(o_sb[:], pso[:])
                    nc.sync.dma_start(
                        out[qt0 * P:(qt0 + 1) * P,
                            nh * QB:(nh + 1) * QB],
                        o_sb[:],
                    )

        # ---- interleaved emission schedule ----
        # PE queue order == execution order. Scores blocks are
        # ACT-throttled (~8.3us each via the pss double-buffer), so the
        # Q/K/V projection matmuls placed between them execute when
        # their DMA lands, filling PE gaps. attnV trails scores by 3-4
        # blocks (e2 pool bufs=5).
        pss_p = st.enter_context(tc.tile_pool(name="ps_s", bufs=2,
                                              space="PSUM"))
        with tc.tile_pool(name="ps_kq", bufs=4, space="PSUM") as ps_k:
            nc.sync.dma_start(wk_sb[:], wk_r)
            # PE warm-up: ramp the tensor engine to full clock on junk
            # matmuls over wk while xkb0 streams, and prefetch the exp
            # activation table, so Kb0 runs at full rate immediately.
            nc.sync.dma_start(bk_sb[:], bk[:])
            nc.sync.dma_start(ones_sb[:], ones_row[:])
            wps = pss_p.tile([P, 2 * QB], F32, tag="s", name="warm_ps")
            for w in range(2):
                nc.tensor.matmul(wps[:, :QB], wk_sb[:, 0, :P],
                                 wk_sb[:, 0:2, :], start=True, stop=True)
            nc.scalar.activation(warm[:], wk_sb[0:1, 0:1, 0:1],
                                 mybir.ActivationFunctionType.Exp)
            emit_kblock(0, ps_k, act_p1=True)
            nc.sync.dma_start(wq_sb[:], wq_r)
            nc.sync.dma_start(bq_sb[:], bq[:])
            q0_p1 = emit_qblock(0, ps_k, defer_drains="p1")
            Sblk(0, 0, range(0, 2))
            emit_kblock(1, ps_k, act_p1=True)
            q0_p1()
            Sblk(0, 0, range(2, 4))
            k2_h1 = emit_kblock(2, ps_k, act_p1=True, defer_p0h1=True)
            Sblk(0, 0, range(4, 6))
            k3_h1 = emit_kblock(3, ps_k, act_p1=True, defer_p0h1=True)
            Sblk(0, 0, range(6, 8))
            emit_qblock(1, ps_k)
            k2_h1()
            k3_h1()
        q_psum = q_stack.enter_context(
            tc.tile_pool(name="ps_q", bufs=2, space="PSUM"))
        Sblk(0, 1)
        emit_qblock(2, q_psum)
        Sblk(0, 2)
        q3_drains = emit_qblock(3, q_psum, defer_drains=True)
        xvp = v_stack.enter_context(tc.tile_pool(name="xvp", bufs=2))
        wvp = v_stack.enter_context(tc.tile_pool(name="wvp", bufs=1))
        psv_p = v_stack.enter_context(
            tc.tile_pool(name="ps_v", bufs=2, space="PSUM"))
        wv_sb = wvp.tile([P, DC, CPC], BF)
        nc.sync.dma_start(wv_sb[:], wv.rearrange("(dc p) c -> p dc c", p=P))
        nc.sync.dma_start(bv_sb[:], bv[:])
        nc.sync.dma_start(ones_sb[:], ones_row[:])
        emit_vblock(0, xvp, psv_p)
        emit_vblock(1, xvp, psv_p)
        Sblk(0, 3)
        emit_vblock(2, xvp, psv_p)
        emit_vblock(3, xvp, psv_p)
        Sblk(1, 0)
        emit_vblock(4, xvp, psv_p)
        emit_vblock(5, xvp, psv_p)
        Sblk(1, 1)
        emit_vblock(6, xvp, psv_p)
        emit_vblock(7, xvp, psv_p)
        with nc.allow_non_contiguous_dma(
                reason="one-time 16KB ones-column init"):
            nc.sync.dma_start(
                v_sb[:, :, :, DK:DK + 1],
                vones.rearrange("p (kt h) -> p kt h",
                                kt=KT, h=HPC)[:, :, :, None],
            )
        nc.sync.dma_start(wo_sb[:],
                          wo.rearrange("(dj p) n -> p dj n", p=P))
        nc.sync.dma_start(ones_fr_sb[:], ones_fr[:])
        q3_drains()
        v_stack.close()
        q_stack.close()
        psa_p = st.enter_context(tc.tile_pool(name="ps_a", bufs=2,
                                              space="PSUM"))
        pso_p = st.enter_context(tc.tile_pool(name="ps_o", bufs=2,
                                              space="PSUM"))

        def Ablk(j, h, pe_bcast=None):
            emit_attnv(j, h, e2t.pop((j, h)), psa_p, pe_bcast=pe_bcast)

        Ablk(0, 0)
        Sblk(1, 2)
        Ablk(0, 1)
        Sblk(1, 3)
        Ablk(0, 2)
        Sblk(2, 0)
        Ablk(0, 3)
        emit_oproj(0, pso_p, chunks=[0])
        Sblk(2, 1)
        emit_oproj(0, pso_p, chunks=[1])
        Ablk(1, 0)
        emit_oproj(0, pso_p, chunks=[2])
        Sblk(2, 2)
        emit_oproj(0, pso_p, chunks=[3])
        Ablk(1, 1)
        Sblk(2, 3)
        Ablk(1, 2)
        Sblk(3, 0)
        Ablk(1, 3)
        emit_oproj(1, pso_p, chunks=[0])
        Sblk(3, 1)
        emit_oproj(1, pso_p, chunks=[1])
        Ablk(2, 0)
        emit_oproj(1, pso_p, chunks=[2])
        Ablk(2, 1)
        emit_oproj(1, pso_p, chunks=[3])
        Sblk(3, 2)
        Ablk(2, 2)
        Ablk(2, 3)
        emit_oproj(2, pso_p, chunks=[0, 1])
        Ablk(3, 0)
        emit_oproj(2, pso_p, chunks=[2, 3])
        Sblk(3, 3)
        Ablk(3, 1)
        Ablk(3, 2)
        Ablk(3, 3, pe_bcast=pso_p)
        emit_oproj(3, pso_p, act_copy=True, alt_pool=psa_p)

    nc.compile()
    return nc


_PROGRAM_CACHE = {}


def _get_program(seq=S):
    if seq not in _PROGRAM_CACHE:
        _PROGRAM_CACHE[seq] = build_program(seq)
    return _PROGRAM_CACHE[seq]


def make_in_maps(queries, keys, values, Wq, bq, Wk, bk, Wv, bv, Wo, bo):
    """Per-core input dicts implementing the sharding (bf16 on device)."""
    f32 = np.float32
    bf16 = ml_dtypes.bfloat16
    seq = np.asarray(queries).shape[1]
    xT = {}
    for b in range(B):
        xT[b] = tuple(
            np.ascontiguousarray(
                np.asarray(a[b], dtype=f32).T.astype(bf16))
            for a in (queries, keys, values)
        )
    Wq, Wk, Wv, Wo = (np.asarray(a, dtype=f32) for a in (Wq, Wk, Wv, Wo))
    bq, bk, bv = (np.asarray(a, dtype=f32) for a in (bq, bk, bv))
    in_maps = []
    for c in range(NCORES):
        b, g = divmod(c, GROUPS)
        cs = slice(g * CPC, (g + 1) * CPC)
        qT, kT, vT = xT[b]
        in_maps.append({
            "xqT": qT, "xkT": kT, "xvT": vT,
            "wq": np.ascontiguousarray(Wq[:, cs].astype(bf16)),
            "wk": np.ascontiguousarray(Wk[:, cs].astype(bf16)),
            "wv": np.ascontiguousarray(Wv[:, cs].astype(bf16)),
            "wo": np.ascontiguousarray(Wo[cs, :].astype(bf16)),
            "bq": np.ascontiguousarray(bq[cs].astype(bf16))[None, :],
            "bk": np.ascontiguousarray(bk[cs].astype(bf16))[None, :],
            "bv": np.ascontiguousarray(bv[cs].astype(bf16))[None, :],
            "ones_row": np.ones((1, QB), dtype=bf16),
            "ones_fr": np.ones((1, DK), dtype=f32),
            "vones": np.ones((P, (seq // P) * HPC), dtype=bf16),
        })
    return in_maps


def combine_outputs(results, bo):
    """Host all-reduce of the Wo row-shard partials + bias."""
    bo = np.asarray(bo, dtype=np.float32)
    outs = []
    for b in range(B):
        acc = results[b * GROUPS]["out"].astype(np.float32).copy()
        for g in range(1, GROUPS):
            acc += results[b * GROUPS + g]["out"]
        outs.append(acc + bo)
    return np.stack(outs)


def kernel(queries, keys, values, Wq, bq, Wk, bk, Wv, bv, Wo, bo):
    nc = _get_program()
    in_maps = make_in_maps(queries, keys, values, Wq, bq, Wk, bk, Wv, bv,
                           Wo, bo)
    res = run_bass_kernel_spmd(nc, in_maps, list(range(NCORES)))
    return combine_outputs(res.results, bo)


# revision 52
# speedup vs baseline: 1.0163x; 1.0020x over previous
"""MultiHeadAttention forward on 8 Trainium2 NeuronCores.

Sharding (Megatron-style tensor parallel x data parallel):
  core c (0..7): batch b = c // 4, head group g = c % 4 (4 of 16 heads).
  Wq/Wk/Wv column-sharded ([1024, 256] per core), Wo row-sharded
  ([256, 1024] per core). Each core computes a partial output
  [S, D] = attn(heads g) @ Wo_rows; the host sums the 4 partials per
  batch and adds bo (the "all-reduce" runs on host since full outputs
  are gathered anyway).

Projections/attnV/O run in bf16 (inputs converted on host; f32 PSUM
accumulate), halving HBM traffic vs f32r at the same PE rate. The
scores matmul runs in fp8-e4m3 DoubleRow perf mode (2x PE rate): K^T
and Q^T are drained from their projection psums into a [32, 2, s]
layout (d = 32*i + p) so each head's QK^T contracts as two 32-row
halves summed in the PE. Measured end-to-end rel err ~7.8e-3 vs the
2e-2 gate.

Schedule notes (the three serial chains that matter):
  - ACT runs the 128 softmax exps (~133us serial) plus a few psum
    drains placed in its natural stalls; it is kept fed from ~22us on.
  - Projection psums add biases via ones-row matmuls INSIDE the psum
    accumulation so the psum->SBUF drains are pure copies with no DMA
    dependency (the conservative DMA-queue semaphore encoding would
    otherwise stall each drain on every earlier DMA on its queue).
  - PE is warmed up on junk matmuls during the first DMA so Kb0 runs
    at full clock; K streams block-major so the first scores chunk
    only needs K-block 0 + Q-block 0.
  - attnV trails scores by ~4 blocks (e2 pool bufs=6); softmax
    normalization = DVE reciprocal + GPSIMD partition-broadcast (Pool
    engine, otherwise idle) + one DVE multiply.
  - O projection drains via DVE mid-kernel and via ACT for the last
    q-block (ACT is idle once the exps finish); host sums the 4
    row-shard partials per batch and adds bo.
"""

import math
from contextlib import ExitStack

import numpy as np
import ml_dtypes

import concourse.bass as bass
import concourse.mybir as mybir
import concourse.tile as tile
from concourse import bacc
from concourse.bass_utils import run_bass_kernel_spmd

P = 128
B, S, D, H = 2, 2048, 1024, 16
NCORES = 8
GROUPS = NCORES // B          # 4 head-groups
HPC = H // GROUPS             # 4 heads per core
DK = D // H                   # 64
CPC = HPC * DK                # 256 cols per core
NP = CPC // P                 # 2 head pairs per core
DC = D // P                   # 8 contraction chunks over D
QB = 512                      # q block (matmul moving free dim)

F32 = mybir.dt.float32
BF = mybir.dt.bfloat16
F8 = mybir.dt.float8e4


def build_program(seq=S):
    KT = seq // P             # k tiles
    NJ = seq // QB            # q blocks
    K2 = KT // 2              # two score k-tiles share one psum / exp op
    KT2 = KT // 2             # k-tile pairs for the V projection
    inv_sqrt_s = 1.0 / math.sqrt(S)  # reference scales by sqrt(full S)

    nc = bacc.Bacc("TRN2", target_bir_lowering=False, debug=False,
                   num_devices=NCORES)
    xqT = nc.declare_dram_parameter("xqT", [D, seq], BF, isOutput=False)
    xkT = nc.declare_dram_parameter("xkT", [D, seq], BF, isOutput=False)
    xvT = nc.declare_dram_parameter("xvT", [D, seq], BF, isOutput=False)
    wq = nc.declare_dram_parameter("wq", [D, CPC], BF, isOutput=False)
    wk = nc.declare_dram_parameter("wk", [D, CPC], BF, isOutput=False)
    wv = nc.declare_dram_parameter("wv", [D, CPC], BF, isOutput=False)
    wo = nc.declare_dram_parameter("wo", [CPC, D], BF, isOutput=False)
    bq = nc.declare_dram_parameter("bq", [1, CPC], BF, isOutput=False)
    bk = nc.declare_dram_parameter("bk", [1, CPC], BF, isOutput=False)
    bv = nc.declare_dram_parameter("bv", [1, CPC], BF, isOutput=False)
    ones_row = nc.declare_dram_parameter("ones_row", [1, QB], BF,
                                         isOutput=False)
    ones_fr = nc.declare_dram_parameter("ones_fr", [1, DK],
                                        mybir.dt.float32r, isOutput=False)
    vones = nc.declare_dram_parameter("vones", [P, KT * HPC], BF,
                                      isOutput=False)
    out = nc.declare_dram_parameter("out", [seq, D], BF, isOutput=True)

    xqT_r = xqT.rearrange("(dc p) s -> p dc s", p=P)
    xkT_r = xkT.rearrange("(dc p) s -> p dc s", p=P)
    xvT_r = xvT.rearrange("(dc p) s -> p dc s", p=P)

    with tile.TileContext(nc) as tc, ExitStack() as st:
        consts = st.enter_context(tc.tile_pool(name="consts", bufs=1))
        bq_sb = consts.tile([1, CPC], BF)
        bk_sb = consts.tile([1, CPC], BF)
        bv_sb = consts.tile([1, CPC], BF)
        ones_sb = consts.tile([1, QB], BF)
        ones_fr_sb = consts.tile([1, DK], mybir.dt.float32r)

        # Persistent activations. K^T/Q^T live in fp8 with the
        # DoubleRow layout: head h on partitions 32h..32h+31, free dims
        # (i, s) where d = 32 i + p — so the scores matmul runs in fp8
        # DoubleRow perf mode at 0.5 cycles/row (2x PE rate).
        # matmul operands need base partition in {0, 32, 64}: heads
        # 0-2 share tile 0 at bases 0/32/64, head 3 gets tile 1 base 0.
        kt_f8 = [consts.tile([P, 2, seq], F8, name=f"kt_f8_{t}")
                 for t in range(2)]
        qt_f8 = [[consts.tile([P, 2, QB], F8, name=f"qt_f8_{j}_{t}")
                  for t in range(2)] for j in range(NJ)]

        def hrow(h):
            t, b = (0, 32 * h) if h < 3 else (1, 0)
            return t, slice(b, b + 32)
        v_sb = consts.tile([P, KT, HPC, DK + 1], BF)
        at_j = [consts.tile([P, NP, QB], BF, name=f"at_j{j}")
                for j in range(NJ)]
        wo_sb = consts.tile([P, NP, D], BF)

        # Warm-up exp so the activation-table load happens during the
        # initial DMA instead of right before the first scores exp.
        warm = consts.tile([1, 1], F32)
        # biases are added inside the psum accumulation via a ones-row
        # matmul (like V) so the psum drains carry NO DMA dependency:
        # the conservative DMA-queue semaphore encoding would otherwise
        # stall each drain on every earlier-enqueued DMA on that queue.

        wqp = st.enter_context(tc.tile_pool(name="wqp", bufs=1))
        xqp = st.enter_context(tc.tile_pool(name="xqp", bufs=2))
        wq_sb = wqp.tile([P, DC, CPC], BF)
        wq_r = wq.rearrange("(dc p) c -> p dc c", p=P)

        def emit_qblock(j, pool, defer_drains=False, split_dma=False):
            xt = xqp.tile([P, DC, QB], BF, tag="xq")
            qsl = xqT_r[:, :, j * QB:(j + 1) * QB]
            if split_dma:
                # halve the first block's DMA so dc 0-3 matmuls start a
                # transfer earlier during the serial bootstrap
                nc.sync.dma_start(xt[:, 0:DC // 2], qsl[:, 0:DC // 2])
                nc.sync.dma_start(xt[:, DC // 2:], qsl[:, DC // 2:])
            else:
                nc.sync.dma_start(xt[:], qsl)
            ps = [pool.tile([P, QB], F32, tag="k", name=f"psq_{j}_{pi}")
                  for pi in range(NP)]
            for dc in range(DC):
                for pi in range(NP):
                    nc.tensor.matmul(
                        ps[pi][:],
                        wq_sb[:, dc, pi * P:(pi + 1) * P],
                        xt[:, dc],
                        start=(dc == 0), stop=False,
                    )
            for pi in range(NP):
                nc.tensor.matmul(  # += bq^T @ ones  (bias add)
                    ps[pi][:], bq_sb[:, pi * P:(pi + 1) * P], ones_sb[:],
                    start=False, stop=True,
                )

            def drains(pis):
                for pi in pis:
                    for hp in range(2):
                        for i in range(2):
                            r = hp * 64 + 32 * i
                            t, rows = hrow(2 * pi + hp)
                            nc.vector.tensor_copy(
                                qt_f8[j][t][rows, i, :],
                                ps[pi][r:r + 32, :])
            if defer_drains == "p1":
                drains([0])
                return lambda: drains([1])
            if defer_drains:
                return lambda: drains(range(NP))
            drains(range(NP))

        # K projection, block-major like Q (one [P, DC, QB] DMA per
        # block) so kt_p columns drain progressively and the first
        # scores exps can start right after K's matmuls.
        xkp = st.enter_context(tc.tile_pool(name="xkp", bufs=2))
        wkp = st.enter_context(tc.tile_pool(name="wkp", bufs=1))
        wk_sb = wkp.tile([P, DC, CPC], BF)
        wk_r = wk.rearrange("(dc p) c -> p dc c", p=P)

        def emit_kblock(qc, pool, act_p1=False, split_dma=False,
                        defer_p0h1=False):
            xt = xkp.tile([P, DC, QB], BF, tag="xk")
            ksl = xkT_r[:, :, qc * QB:(qc + 1) * QB]
            if split_dma:
                nc.sync.dma_start(xt[:, 0:DC // 2], ksl[:, 0:DC // 2])
                nc.sync.dma_start(xt[:, DC // 2:], ksl[:, DC // 2:])
            else:
                nc.sync.dma_start(xt[:], ksl)
            ps = [pool.tile([P, QB], F32, tag="k", name=f"psk_{qc}_{pi}")
                  for pi in range(NP)]
            for dc in range(DC):
                for pi in range(NP):
                    nc.tensor.matmul(
                        ps[pi][:],
                        wk_sb[:, dc, pi * P:(pi + 1) * P],
                        xt[:, dc],
                        start=(dc == 0), stop=False,
                    )
            for pi in range(NP):
                nc.tensor.matmul(  # += bk^T @ ones  (bias add)
                    ps[pi][:], bk_sb[:, pi * P:(pi + 1) * P], ones_sb[:],
                    start=False, stop=True,
                )
            def kdrain(pi, hp):
                for i in range(2):
                    r = hp * 64 + 32 * i
                    t, rows = hrow(2 * pi + hp)
                    dst = kt_f8[t][rows, i, qc * QB:(qc + 1) * QB]
                    if pi == 1 and act_p1:
                        # heads 2/3 drain on ACT: they are needed a
                        # whole exp-block later and fill the early
                        # ACT stalls, lightening the serial DVE queue
                        nc.scalar.copy(dst, ps[pi][r:r + 32, :])
                    else:
                        nc.vector.tensor_copy(dst, ps[pi][r:r + 32, :])
            kdrain(0, 0)
            if not defer_p0h1:
                kdrain(0, 1)
            kdrain(1, 0)
            kdrain(1, 1)
            if defer_p0h1:
                return lambda: kdrain(0, 1)

        # ---- attention pipeline pieces ----
        ep = st.enter_context(tc.tile_pool(name="epool", bufs=6))
        rp = st.enter_context(tc.tile_pool(name="rpool", bufs=2))
        op = st.enter_context(tc.tile_pool(name="opool", bufs=8))

        def emit_scores(j, h, e2, k2s, pss_p):
            t, rows = hrow(h)
            for k2 in k2s:
                pss = pss_p.tile([P, 2 * QB], F32, tag="s",
                                 name=f"pss_{j}_{h}_{k2}")
                for half in range(2):
                    kt = 2 * k2 + half
                    nc.tensor.matmul(
                        pss[:, half * QB:(half + 1) * QB],
                        kt_f8[t][rows, :, kt * P:(kt + 1) * P],
                        qt_f8[j][t][rows, :, :],
                        start=True, stop=True,
                        perf_mode=mybir.MatmulPerfMode.DoubleRow,
                    )
                nc.scalar.activation(
                    e2[:, k2], pss[:],
                    mybir.ActivationFunctionType.Exp,
                    scale=inv_sqrt_s,
                )

        e2t = {}

        def Sblk(j, h, k2s=None, pss_pool=None):
            if k2s is None or k2s[0] == 0:
                e2 = ep.tile([P, K2, 2 * QB], BF, tag="E",
                             name=f"e2_{j}_{h}")
                e2t[(j, h)] = e2
            emit_scores(j, h, e2t[(j, h)], k2s or range(K2),
                        pss_pool or pss_p)

        # ---- V projection (kt-pair-major, one psum bank per pair) ----
        v_stack = ExitStack()
        q_stack = ExitStack()

        def emit_vblock(kt2, xvp, psv_p):
            xt = xvp.tile([P, DC, 2 * P], BF, tag="xv")
            nc.sync.dma_start(
                xt[:], xvT_r[:, :, kt2 * 2 * P:(kt2 + 1) * 2 * P])
            psv = psv_p.tile([P, 2, CPC], F32, tag="v", name=f"psv_{kt2}")
            for dc in range(DC):
                for half in range(2):
                    nc.tensor.matmul(
                        psv[:, half],
                        xt[:, dc, half * P:(half + 1) * P],
                        wv_sb[:, dc],
                        start=(dc == 0 and half == 0), stop=False,
                    )
            for half in range(2):
                nc.tensor.matmul(  # += ones^T @ bv  (bias add)
                    psv[:, half], ones_sb[:, :P], bv_sb[:],
                    start=False, stop=(half == 1),
                )
            for half in range(2):
                nc.vector.tensor_copy(
                    v_sb[:, 2 * kt2 + half, :, 0:DK],
                    psv[:, half].rearrange("p (h d) -> p h d", h=HPC),
                )

        def emit_attnv(j, h, e2, psa_p, pe_bcast=None):
            hp, hj = h % 2, h // 2
            prow = slice(hp * DK, (hp + 1) * DK)
            psa = psa_p.tile([P, QB], F32, tag="a", name=f"psa_{j}_{h}")
            for kt in range(KT):
                nc.tensor.matmul(
                    psa[:DK + 1],
                    v_sb[:, kt, h, :],
                    e2[:, kt // 2, (kt % 2) * QB:(kt % 2 + 1) * QB],
                    start=(kt == 0), stop=(kt == KT - 1),
                )
            # softmax denominator is psa row DK; normalize via DVE recip +
            # GPSIMD partition-broadcast (Pool engine) + DVE multiply.
            # On the last block the Pool round-trip is on the critical
            # tail: broadcast via a PE matmul (f32r, exact) into a
            # borrowed psum bank instead.
            if pe_bcast is not None:
                rc = rp.tile([1, QB], mybir.dt.float32r, tag="rcr",
                             bufs=1)
                with nc.allow_low_precision(
                        reason="f32r reciprocal for matmul bcast"):
                    nc.vector.reciprocal(rc[:], psa[DK:DK + 1, :])
                prc = pe_bcast.tile([P, QB], F32, tag="o",
                                    name=f"prc_{j}_{h}")
                nc.tensor.matmul(prc[:DK], ones_fr_sb[:], rc[:],
                                 start=True, stop=True)
                # tensor_tensor may read only one PSUM operand: stage
                # psa through SBUF (overlaps the PE broadcast matmul)
                atmp = rp.tile([DK, QB], F32, tag="prc")
                nc.vector.tensor_copy(atmp[:], psa[:DK])
                nc.vector.tensor_tensor(
                    at_j[j][prow, hj, :], atmp[:], prc[:DK],
                    mybir.AluOpType.mult,
                )
                return
            rc = rp.tile([1, QB], F32, tag="rc")
            nc.vector.reciprocal(rc[:], psa[DK:DK + 1, :])
            prc = rp.tile([DK, QB], F32, tag="prc")
            nc.gpsimd.partition_broadcast(prc[:], rc[:])
            nc.vector.tensor_tensor(
                at_j[j][prow, hj, :], psa[:DK], prc[:],
                mybir.AluOpType.mult,
            )

        def emit_oproj(j, pso_p, act_copy=False, chunks=None,
                       alt_pool=None):
            for ql in (range(QB // P) if chunks is None else chunks):
                qt0 = j * (QB // P) + ql
                for nh in range(D // QB):
                    o_sb = op.tile([P, QB], BF, tag="o_sb",
                                   name=f"osb_{qt0}_{nh}")
                    # tail block: borrow the attnV psum banks (drained
                    # by then) so four banks rotate instead of two
                    pp = alt_pool if (alt_pool is not None
                                      and (ql * 2 + nh) % 2 == 1) else pso_p
                    pso = pp.tile([P, QB], F32,
                                  tag="a" if pp is alt_pool else "o",
                                  name=f"pso_{qt0}_{nh}")
                    for dj in range(NP):
                        nc.tensor.matmul(
                            pso[:],
                            at_j[j][:, dj, ql * P:(ql + 1) * P],
                            wo_sb[:, dj, nh * QB:(nh + 1) * QB],
                            start=(dj == 0), stop=(dj == NP - 1),
                        )
                    if act_copy and (ql * 2 + nh) % 2 == 0:
                        # tail block: alternate ACT/DVE copies (both are
                        # idle once the exps and norms finish)
                        nc.scalar.copy(o_sb[:], pso[:])
                    else:
                        nc.vector.tensor_copy(o_sb[:], pso[:])
                    nc.sync.dma_start(
                        out[qt0 * P:(qt0 + 1) * P,
                            nh * QB:(nh + 1) * QB],
                        o_sb[:],
                    )

        # ---- interleaved emission schedule ----
        # PE queue order == execution order. Scores blocks are
        # ACT-throttled (~8.3us each via the pss double-buffer), so the
        # Q/K/V projection matmuls placed between them execute when
        # their DMA lands, filling PE gaps. attnV trails scores by 3-4
        # blocks (e2 pool bufs=5).
        pss_p = st.enter_context(tc.tile_pool(name="ps_s", bufs=2,
                                              space="PSUM"))
        with tc.tile_pool(name="ps_kq", bufs=4, space="PSUM") as ps_k:
            nc.sync.dma_start(wk_sb[:], wk_r)
            # PE warm-up: ramp the tensor engine to full clock on junk
            # matmuls over wk while xkb0 streams, and prefetch the exp
            # activation table, so Kb0 runs at full rate immediately.
            nc.sync.dma_start(bk_sb[:], bk[:])
            nc.sync.dma_start(ones_sb[:], ones_row[:])
            wps = pss_p.tile([P, 2 * QB], F32, tag="s", name="warm_ps")
            for w in range(1):
                nc.tensor.matmul(wps[:, :QB], wk_sb[:, 0, :P],
                                 wk_sb[:, 0:2, :], start=True, stop=True)
            nc.scalar.activation(warm[:], wk_sb[0:1, 0:1, 0:1],
                                 mybir.ActivationFunctionType.Exp)
            emit_kblock(0, ps_k, act_p1=True)
            nc.sync.dma_start(wq_sb[:], wq_r)
            nc.sync.dma_start(bq_sb[:], bq[:])
            q0_p1 = emit_qblock(0, ps_k, defer_drains="p1")
            Sblk(0, 0, range(0, 2))
            emit_kblock(1, ps_k, act_p1=True)
            q0_p1()
            Sblk(0, 0, range(2, 4))
            k2_h1 = emit_kblock(2, ps_k, act_p1=True, defer_p0h1=True)
            Sblk(0, 0, range(4, 6))
            k3_h1 = emit_kblock(3, ps_k, act_p1=True, defer_p0h1=True)
            Sblk(0, 0, range(6, 8))
            emit_qblock(1, ps_k)
            k2_h1()
            k3_h1()
        q_psum = q_stack.enter_context(
            tc.tile_pool(name="ps_q", bufs=2, space="PSUM"))
        Sblk(0, 1)
        emit_qblock(2, q_psum)
        Sblk(0, 2)
        q3_drains = emit_qblock(3, q_psum, defer_drains=True)
        xvp = v_stack.enter_context(tc.tile_pool(name="xvp", bufs=2))
        wvp = v_stack.enter_context(tc.tile_pool(name="wvp", bufs=1))
        psv_p = v_stack.enter_context(
            tc.tile_pool(name="ps_v", bufs=2, space="PSUM"))
        wv_sb = wvp.tile([P, DC, CPC], BF)
        nc.sync.dma_start(wv_sb[:], wv.rearrange("(dc p) c -> p dc c", p=P))
        nc.sync.dma_start(bv_sb[:], bv[:])
        nc.sync.dma_start(ones_sb[:], ones_row[:])
        emit_vblock(0, xvp, psv_p)
        emit_vblock(1, xvp, psv_p)
        Sblk(0, 3)
        emit_vblock(2, xvp, psv_p)
        emit_vblock(3, xvp, psv_p)
        Sblk(1, 0)
        emit_vblock(4, xvp, psv_p)
        emit_vblock(5, xvp, psv_p)
        Sblk(1, 1)
        emit_vblock(6, xvp, psv_p)
        emit_vblock(7, xvp, psv_p)
        with nc.allow_non_contiguous_dma(
                reason="one-time 16KB ones-column init"):
            nc.sync.dma_start(
                v_sb[:, :, :, DK:DK + 1],
                vones.rearrange("p (kt h) -> p kt h",
                                kt=KT, h=HPC)[:, :, :, None],
            )
        nc.sync.dma_start(wo_sb[:],
                          wo.rearrange("(dj p) n -> p dj n", p=P))
        nc.sync.dma_start(ones_fr_sb[:], ones_fr[:])
        q3_drains()
        v_stack.close()
        q_stack.close()
        psa_p = st.enter_context(tc.tile_pool(name="ps_a", bufs=2,
                                              space="PSUM"))
        pso_p = st.enter_context(tc.tile_pool(name="ps_o", bufs=2,
                                              space="PSUM"))

        def Ablk(j, h, pe_bcast=None):
            emit_attnv(j, h, e2t.pop((j, h)), psa_p, pe_bcast=pe_bcast)

        Ablk(0, 0)
        Sblk(1, 2)
        Ablk(0, 1)
        Sblk(1, 3)
        Ablk(0, 2)
        Sblk(2, 0)
        Ablk(0, 3)
        emit_oproj(0, pso_p, chunks=[0])
        Sblk(2, 1)
        emit_oproj(0, pso_p, chunks=[1])
        Ablk(1, 0)
        emit_oproj(0, pso_p, chunks=[2])
        Sblk(2, 2)
        emit_oproj(0, pso_p, chunks=[3])
        Ablk(1, 1)
        Sblk(2, 3)
        Ablk(1, 2)
        Sblk(3, 0)
        Ablk(1, 3)
        emit_oproj(1, pso_p, chunks=[0])
        Sblk(3, 1)
        emit_oproj(1, pso_p, chunks=[1])
        Ablk(2, 0)
        emit_oproj(1, pso_p, chunks=[2])
        Ablk(2, 1)
        emit_oproj(1, pso_p, chunks=[3])
        Sblk(3, 2)
        Ablk(2, 2)
        Ablk(2, 3)
        emit_oproj(2, pso_p, chunks=[0, 1])
        Ablk(3, 0)
        emit_oproj(2, pso_p, chunks=[2, 3])
        Sblk(3, 3)
        Ablk(3, 1)
        Ablk(3, 2)
        Ablk(3, 3, pe_bcast=pso_p)
        emit_oproj(3, pso_p, act_copy=True, alt_pool=psa_p)

    nc.compile()
    return nc


_PROGRAM_CACHE = {}


def _get_program(seq=S):
    if seq not in _PROGRAM_CACHE:
        _PROGRAM_CACHE[seq] = build_program(seq)
    return _PROGRAM_CACHE[seq]


def make_in_maps(queries, keys, values, Wq, bq, Wk, bk, Wv, bv, Wo, bo):
    """Per-core input dicts implementing the sharding (bf16 on device)."""
    f32 = np.float32
    bf16 = ml_dtypes.bfloat16
    seq = np.asarray(queries).shape[1]
    xT = {}
    for b in range(B):
        xT[b] = tuple(
            np.ascontiguousarray(
                np.asarray(a[b], dtype=f32).T.astype(bf16))
            for a in (queries, keys, values)
        )
    Wq, Wk, Wv, Wo = (np.asarray(a, dtype=f32) for a in (Wq, Wk, Wv, Wo))
    bq, bk, bv = (np.asarray(a, dtype=f32) for a in (bq, bk, bv))
    in_maps = []
    for c in range(NCORES):
        b, g = divmod(c, GROUPS)
        cs = slice(g * CPC, (g + 1) * CPC)
        qT, kT, vT = xT[b]
        in_maps.append({
            "xqT": qT, "xkT": kT, "xvT": vT,
            "wq": np.ascontiguousarray(Wq[:, cs].astype(bf16)),
            "wk": np.ascontiguousarray(Wk[:, cs].astype(bf16)),
            "wv": np.ascontiguousarray(Wv[:, cs].astype(bf16)),
            "wo": np.ascontiguousarray(Wo[cs, :].astype(bf16)),
            "bq": np.ascontiguousarray(bq[cs].astype(bf16))[None, :],
            "bk": np.ascontiguousarray(bk[cs].astype(bf16))[None, :],
            "bv": np.ascontiguousarray(bv[cs].astype(bf16))[None, :],
            "ones_row": np.ones((1, QB), dtype=bf16),
            "ones_fr": np.ones((1, DK), dtype=f32),
            "vones": np.ones((P, (seq // P) * HPC), dtype=bf16),
        })
    return in_maps


def combine_outputs(results, bo):
    """Host all-reduce of the Wo row-shard partials + bias."""
    bo = np.asarray(bo, dtype=np.float32)
    outs = []
    for b in range(B):
        acc = results[b * GROUPS]["out"].astype(np.float32).copy()
        for g in range(1, GROUPS):
            acc += results[b * GROUPS + g]["out"]
        outs.append(acc + bo)
    return np.stack(outs)


def kernel(queries, keys, values, Wq, bq, Wk, bk, Wv, bv, Wo, bo):
    nc = _get_program()
    in_maps = make_in_maps(queries, keys, values, Wq, bq, Wk, bk, Wv, bv,
                           Wo, bo)
    res = run_bass_kernel_spmd(nc, in_maps, list(range(NCORES)))
    return combine_outputs(res.results, bo)
